# revision 3
# baseline (speedup 1.0000x reference)
"""FPN + RPN detector kernel for Trainium2, 8-core SPMD.

Sharding: core = (image b in {0,1}) x (row-quarter q in {0..3}). Host slices
inputs with halos (zero-padded), every core runs the same program on its
slice, host crops/reassembles. No cross-core communication.

All convs are bf16 matmuls (full PE rate at any N, fast weight load),
channels on partitions in two halves of 128, PSUM f32 accumulation over
taps/halves; biases and outputs stay f32.

The 3x3 convs at L2/L3/L4 (tr + rpn1) use 1-D Winograd F(2,3) along the
width: per 2 output cols, 4 transform points x 3 dy x 2 ih = 24 matmul
cycles/2px vs 36 direct (1.5x less PE work). Input transform V on DVE,
output transform A^T on DVE/ACT with fused bias. L5/L6 stay direct.
"""
import numpy as np

N_CORES = 8

# Quarter starts chosen so every upsample/subsample alignment offset is
# uniform across cores (SPMD requirement).
S2 = [0, 48, 96, 144]
S3 = [0, 24, 48, 72]
S4 = [0, 12, 24, 36]
S5 = [0, 6, 12, 18]
S6 = [0, 3, 6, 9]
H = [200, 100, 50, 25, 13]
W = [200, 100, 50, 25, 13]
OWN = [56, 28, 14, 7, 4]
VALID = [
    [48, 48, 48, 56],
    [24, 24, 24, 28],
    [12, 12, 12, 14],
    [6, 6, 6, 7],
    [3, 3, 3, 4],
]
M_ROWS = [60, 32, 18, 13]     # m2..m5 region rows (m_l from S_l-2, m5 S5-3)
CIN = [256, 512, 1024, 2048]
KH = [2, 4, 8, 16]
CW = [200, 100, 50, 26]       # c-slice widths (c5 x-padded to 26)
CONVW = [200, 100, 50, 26, 14]  # conv window x-counts (even)

_CACHED = {}


def _build(reps=1):
    import concourse.bacc as bacc
    import concourse.mybir as mybir
    from concourse.tile import TileContext

    F32, BF16 = mybir.dt.float32, mybir.dt.bfloat16
    AF = mybir.ActivationFunctionType
    ALU = mybir.AluOpType

    nc = bacc.Bacc("TRN2", target_bir_lowering=False, debug=False,
                   num_devices=N_CORES)

    c_d = [nc.dram_tensor(f"c{i}", [KH[i], 128, M_ROWS[i], CW[i]], BF16,
                          kind="ExternalInput") for i in range(4)]
    wlat_d = [nc.dram_tensor(f"wlat{i}", [KH[i], 128, 2, 128], BF16,
                             kind="ExternalInput") for i in range(4)]
    # direct 3x3 weights (L5 tr conv + L5/L6 rpn conv)
    wtr3_d = nc.dram_tensor("wtr3", [2, 128, 2, 3, 3, 128], BF16,
                            kind="ExternalInput")
    wrpn1_d = nc.dram_tensor("wrpn1", [2, 128, 2, 3, 3, 128], BF16,
                             kind="ExternalInput")
    # Winograd F(2,3) width-transformed weights (t axis of 4)
    wtrU_d = [nc.dram_tensor(f"wtrU{i}", [2, 128, 2, 3, 4, 128], BF16,
                             kind="ExternalInput") for i in range(3)]
    wrpnU_d = nc.dram_tensor("wrpnU", [2, 128, 2, 3, 4, 128], BF16,
                             kind="ExternalInput")
    wrpn2_d = nc.dram_tensor("wrpn2", [2, 128, 16], BF16,
                             kind="ExternalInput")
    btr_d = nc.dram_tensor("btr", [4, 128, 2], F32, kind="ExternalInput")
    brpn1_d = nc.dram_tensor("brpn1", [128, 2], F32, kind="ExternalInput")
    brpn2_d = nc.dram_tensor("brpn2", [16, 1], F32, kind="ExternalInput")
    zeros_d = nc.dram_tensor("zeros", [128, 512], BF16, kind="ExternalInput")
    masks_d = nc.dram_tensor("masks", [128, 244], BF16, kind="ExternalInput")
    MOFF = {}
    _off = 0
    for name, n in [("m2", 60), ("p2", 58), ("m3", 32), ("p3", 30),
                    ("m4", 18), ("p4", 16), ("m5", 13), ("p5", 11),
                    ("p6", 6)]:
        MOFF[name] = _off
        _off += n

    o_d = [nc.dram_tensor(f"o{i}", [OWN[i] * W[i] * 3 * 5], F32,
                          kind="ExternalOutput") for i in range(5)]

    with TileContext(nc, pool_alloc_mode="queue") as tc:
        with (
            tc.tile_pool(name="const", bufs=1) as cpool,
            tc.tile_pool(name="ps", bufs=2, space="PSUM") as pspool,
            tc.tile_pool(name="stg", bufs=1) as stgpool,
            tc.tile_pool(name="escr", bufs=2) as epool,
        ):
            tz = cpool.tile([128, 512], BF16, tag="zeros")
            tmask = cpool.tile([128, 244], BF16, tag="masks")
            twrU = cpool.tile([128, 2, 2, 3, 4, 128], BF16, tag="wrpnU")
            twr2 = cpool.tile([128, 2, 16], BF16, tag="wrpn2")
            tbtr = cpool.tile([128, 4, 2], F32, tag="btr")
            tbr1 = cpool.tile([128, 2], F32, tag="brpn1")
            tbr2 = cpool.tile([16, 1], F32, tag="brpn2")
            nc.gpsimd.dma_start(out=tz[:], in_=zeros_d[:])
            nc.gpsimd.dma_start(out=tmask[:], in_=masks_d[:])
            nc.gpsimd.dma_start(out=tbtr[:],
                                in_=btr_d.rearrange("l p t -> p l t"))
            nc.gpsimd.dma_start(out=tbr1[:], in_=brpn1_d[:])
            nc.gpsimd.dma_start(out=tbr2[:], in_=brpn2_d[:])
            for ih in range(2):
                nc.sync.dma_start(out=twr2[:, ih], in_=wrpn2_d[ih])
                for mo in range(2):
                    nc.sync.dma_start(out=twrU[:, ih, mo],
                                      in_=wrpnU_d[ih, :, mo])

            tm3 = cpool.tile([128, 2, 32, 102], BF16, tag="m3")
            tm4 = cpool.tile([128, 2, 18, 52], BF16, tag="m4")
            tm5 = cpool.tile([128, 2, 13, 28], BF16, tag="m5")

            def zero_cols(t, nrows, cols):
                for ih in range(2):
                    for c0 in cols:
                        nc.vector.tensor_copy(
                            t[:, ih, :, c0].squeeze(), tz[:, :nrows])

            def mask_rows(t, moff, rows, i0, i1):
                for ih in range(2):
                    for r in rows:
                        mk = tmask[:, moff + r:moff + r + 1].broadcast_to(
                            [128, i1 - i0])
                        nc.vector.tensor_tensor(
                            t[:, ih, r, i0:i1], t[:, ih, r, i0:i1], mk,
                            ALU.mult)

            def conv3x3(dst_tile, groups, src_tile, src_row_of, lhsT,
                        bias, relu, lvl, mask_edge=None):
                """Direct 3x3 conv (L5/L6): dst row j reads src rows
                j+src_row_of+dy. relu=True -> unpadded h tile, else padded
                p tile (interior cols 1..W). groups = [(j0, nrows)]."""
                wv, wl = CONVW[lvl], W[lvl]
                for (j0, nr) in groups:
                    for mo in range(2):
                        ps = pspool.tile([128, 4, 512], F32, tag="ps")
                        n = nr * wv
                        first = True
                        for ih in range(2):
                            for dy in range(3):
                                for dx in range(3):
                                    rhs = src_tile[
                                        :, ih,
                                        j0 + src_row_of + dy:
                                        j0 + src_row_of + dy + nr,
                                        dx:dx + wv]
                                    nc.tensor.matmul(
                                        ps[:, 0, :n], lhsT[:, ih, mo, dy, dx],
                                        rhs, start=first,
                                        stop=(ih == 1 and dy == 2 and dx == 2))
                                    first = False
                        psv = ps[:, 0, :n].rearrange("p (r x) -> p r x", x=wv)
                        if relu:
                            dst = dst_tile[:, mo, j0:j0 + nr, :wl]
                            nc.scalar.activation(dst, psv[:, :, :wl], AF.Relu,
                                                 bias=bias[:, mo:mo + 1])
                        else:
                            dst = dst_tile[:, mo, j0:j0 + nr, 1:1 + wl]
                            nc.scalar.activation(dst, psv[:, :, :wl],
                                                 AF.Identity,
                                                 bias=bias[:, mo:mo + 1])
                if mask_edge is not None:
                    nrt, moff = mask_edge
                    mask_rows(dst_tile, moff, [0, 1, nrt - 2, nrt - 1],
                              1, 1 + wl)

            def conv3x3_wino(dst_tile, groups, src_tile, src_row_of, Ut,
                             bias, relu, lvl, vpool, mask_edge=None):
                """1-D Winograd F(2,3) width conv. Same contract as conv3x3.

                V layout [128, ih, 4t, vr, Wg]; per group (j0, nr) and mo,
                P_t accumulates in PSUM bank t; A^T drain:
                  even(2g+1) = P0+P1+P2+b ; odd(2g+2) = P1-P2-P3+b.
                """
                wl = W[lvl]
                wg = wl // 2
                r0 = min(j0 for j0, _ in groups)
                r1 = max(j0 + nr for j0, nr in groups)
                vr = r1 - r0 + 2
                tv = vpool.tile([128, 2, 4, vr, wg], BF16, tag="v")
                sview = src_tile[
                    :, :, src_row_of + r0:src_row_of + r1 + 2,
                    :2 * wg + 2].rearrange("p i r (g s) -> p i r g s", s=2)
                for ih in range(2):
                    d0 = sview[:, ih, :, 0:wg, 0]
                    d1 = sview[:, ih, :, 0:wg, 1]
                    d2 = sview[:, ih, :, 1:wg + 1, 0]
                    d3 = sview[:, ih, :, 1:wg + 1, 1]
                    nc.vector.tensor_tensor(tv[:, ih, 0], d0, d2,
                                            ALU.subtract)
                    nc.vector.tensor_tensor(tv[:, ih, 1], d1, d2, ALU.add)
                    nc.vector.tensor_tensor(tv[:, ih, 2], d2, d1,
                                            ALU.subtract)
                    nc.vector.tensor_tensor(tv[:, ih, 3], d1, d3,
                                            ALU.subtract)
                for (j0, nr) in groups:
                    n = nr * wg
                    for mo in range(2):
                        ps = pspool.tile([128, 4, 512], F32, tag="ps")
                        for t in range(4):
                            for ih in range(2):
                                for dy in range(3):
                                    rhs = tv[:, ih, t,
                                             j0 - r0 + dy:j0 - r0 + dy + nr]
                                    nc.tensor.matmul(
                                        ps[:, t, :n], Ut[:, ih, mo, dy, t],
                                        rhs, start=(ih == 0 and dy == 0),
                                        stop=(ih == 1 and dy == 2))
                        # drains: C1 = P1 + b; even = P0+C1+P2; odd = C1-P3-P2
                        tc1 = epool.tile([128, 512], F32, tag="c1")
                        te = epool.tile([128, 512], F32, tag="e")
                        to = epool.tile([128, 512], F32, tag="o")
                        nc.scalar.activation(tc1[:, :n], ps[:, 1, :n],
                                             AF.Identity,
                                             bias=bias[:, mo:mo + 1])
                        if relu:
                            dv = dst_tile[:, mo, j0:j0 + nr, :2 * wg
                                          ].rearrange(
                                "p r (g s) -> p r g s", s=2)
                            d_ev = dv[:, :, :, 0]
                            d_od = dv[:, :, :, 1]
                        else:
                            dv = dst_tile[:, mo, j0:j0 + nr, :2 * wg + 2
                                          ].rearrange(
                                "p r (g s) -> p r g s", s=2)
                            d_ev = dv[:, :, 0:wg, 1]
                            d_od = dv[:, :, 1:wg + 1, 0]
                        c1v = tc1[:, :n].rearrange("p (r g) -> p r g", g=wg)
                        ev = te[:, :n].rearrange("p (r g) -> p r g", g=wg)
                        ov = to[:, :n].rearrange("p (r g) -> p r g", g=wg)
                        p0 = ps[:, 0, :n].rearrange("p (r g) -> p r g", g=wg)
                        p2 = ps[:, 2, :n].rearrange("p (r g) -> p r g", g=wg)
                        p3 = ps[:, 3, :n].rearrange("p (r g) -> p r g", g=wg)
                        nc.vector.tensor_tensor(ev, p0, c1v, ALU.add)
                        nc.vector.scalar_tensor_tensor(
                            ov, p3, -1.0, c1v, ALU.mult, ALU.add)
                        if relu:
                            tf = epool.tile([128, 512], F32, tag="f")
                            tf2 = epool.tile([128, 512], F32, tag="f2")
                            fv = tf[:, :n].rearrange("p (r g) -> p r g", g=wg)
                            f2v = tf2[:, :n].rearrange("p (r g) -> p r g",
                                                       g=wg)
                            nc.vector.tensor_tensor(fv, ev, p2, ALU.add)
                            nc.vector.scalar_tensor_tensor(
                                f2v, p2, -1.0, ov, ALU.mult, ALU.add)
                            nc.scalar.activation(d_ev, fv, AF.Relu, bias=0.0)
                            nc.scalar.activation(d_od, f2v, AF.Relu, bias=0.0)
                        else:
                            nc.vector.tensor_tensor(d_ev, ev, p2, ALU.add)
                            nc.vector.scalar_tensor_tensor(
                                d_od, p2, -1.0, ov, ALU.mult, ALU.add)
                if mask_edge is not None:
                    nrt, moff = mask_edge
                    mask_rows(dst_tile, moff, [0, 1, nrt - 2, nrt - 1],
                              1, 1 + wl)

            def out_head(h_tile, npix, lvl, px0):
                # o = W2^T h, channel-major: twr2 stationary (16-col weight
                # loads), h streamed in 512-px chunks. Output layout in DRAM
                # is [ch=15, pix] per level; host transposes in _assemble.
                hflat = h_tile.rearrange("p t r x -> p t (r x)")
                tstage = stgpool.tile([16, 2800], F32, tag="ostage")
                for g in range((npix + 511) // 512):
                    p0 = g * 512
                    n = min(512, npix - p0)
                    pso = pspool.tile([128, 4, 512], F32, tag="ps")
                    for ih in range(2):
                        nc.tensor.matmul(
                            pso[:16, 0, :n], twr2[:, ih],
                            hflat[:, ih, p0:p0 + n],
                            start=(ih == 0), stop=(ih == 1))
                    nc.scalar.activation(tstage[:, p0:p0 + n],
                                         pso[:16, 0, :n],
                                         AF.Identity, bias=tbr2[:, 0:1])
                dst = o_d[lvl].rearrange("(c pix) -> c pix", c=15)[
                    :, px0:px0 + npix]
                nc.sync.dma_start(out=dst, in_=tstage[:15, :npix])

            # ================= L5 (+L6) =================
            for _rep in range(reps):
              with tc.tile_pool(name="l5", bufs=1) as pool:
                  tc5 = pool.tile([128, 16, 13, 26], BF16, tag="c5")
                  twl5 = pool.tile([128, 16, 2, 128], BF16, tag="wl5")
                  twt5 = pool.tile([128, 2, 2, 3, 3, 128], BF16, tag="wt5")
                  twr1 = pool.tile([128, 2, 2, 3, 3, 128], BF16, tag="wr1")
                  tp5 = pool.tile([128, 2, 11, 28], BF16, tag="p5")
                  tp6 = pool.tile([128, 2, 6, 16], BF16, tag="p6")
                  th5 = pool.tile([128, 2, 7, 25], BF16, tag="h5")
                  th6 = pool.tile([128, 2, 4, 13], BF16, tag="h6")
                  for kh in range(16):
                      nc.sync.dma_start(out=tc5[:, kh], in_=c_d[3][kh])
                      nc.sync.dma_start(out=twl5[:, kh], in_=wlat_d[3][kh])
                  for ih in range(2):
                      nc.sync.dma_start(out=twt5[:, ih], in_=wtr3_d[ih])
                      for mo in range(2):
                          nc.sync.dma_start(out=twr1[:, ih, mo],
                                            in_=wrpn1_d[ih, :, mo])
                  zero_cols(tm5, 13, [0, 26, 27])
                  zero_cols(tp5, 11, [0, 26, 27])
                  zero_cols(tp6, 6, [0, 14, 15])

                  for mo in range(2):
                      ps = pspool.tile([128, 4, 512], F32, tag="ps")
                      n = 13 * 26
                      for kh in range(16):
                          nc.tensor.matmul(
                              ps[:, 0, :n], twl5[:, kh, mo],
                              tc5[:, kh].rearrange("p r x -> p (r x)"),
                              start=(kh == 0), stop=(kh == 15))
                      psv = ps[:, 0, :n].rearrange("p (r x) -> p r x", x=26)
                      nc.scalar.activation(tm5[:, mo, :, 1:26], psv[:, :, :25],
                                           AF.Copy, bias=0.0)

                  conv3x3(tp5, [(0, 11)], tm5, 0, twt5, tbtr[:, 3], False, 3,
                          mask_edge=(11, MOFF["p5"]))
                  # p6 = p5[::2, ::2]: row j <- p5 row 2j, col x <- p5 col 1+2x
                  for ih in range(2):
                      for j in range(6):
                          src = tp5[:, ih, 2 * j, 1:27].rearrange(
                              "p (x s) -> p x s", s=2)[:, :, 0].squeeze()
                          nc.vector.tensor_copy(tp6[:, ih, j, 1:14], src)
                  conv3x3(th5, [(0, 7)], tp5, 1, twr1, tbr1, True, 3)
                  conv3x3(th6, [(0, 4)], tp6, 0, twr1, tbr1, True, 4)
                  out_head(th5, 7 * 25, 3, 0)
                  out_head(th6, 4 * 13, 4, 0)

              # ================= L4 =================
              with (tc.tile_pool(name="l4", bufs=1) as pool,
                  tc.tile_pool(name="v4", bufs=2) as vpool,
                  tc.tile_pool(name="c4chunk", bufs=3) as c4pool):
                  twl4 = pool.tile([128, 8, 2, 128], BF16, tag="wl4")
                  twt4 = pool.tile([128, 2, 2, 3, 4, 128], BF16, tag="wt4")
                  tp4 = pool.tile([128, 2, 16, 52], BF16, tag="p4")
                  th4 = pool.tile([128, 2, 14, 50], BF16, tag="h4")
                  for kh in range(8):
                      nc.sync.dma_start(out=twl4[:, kh], in_=wlat_d[2][kh])
                  for ih in range(2):
                      nc.sync.dma_start(out=twt4[:, ih], in_=wtrU_d[2][ih])
                  zero_cols(tm4, 18, [0, 51])
                  zero_cols(tp4, 16, [0, 51])

                  for (r0, nr) in [(0, 6), (6, 6), (12, 6)]:
                      tc4 = c4pool.tile([128, 8, 6, 50], BF16, tag="c4")
                      for kh in range(8):
                          nc.sync.dma_start(out=tc4[:, kh],
                                            in_=c_d[2][kh, :, r0:r0 + nr, :])
                      for mo in range(2):
                          ps = pspool.tile([128, 4, 512], F32, tag="ps")
                          n = nr * 50
                          for kh in range(8):
                              rhs = tc4[:, kh].rearrange("p r x -> p (r x)")
                              nc.tensor.matmul(ps[:, 0, :n], twl4[:, kh, mo],
                                               rhs,
                                               start=(kh == 0), stop=(kh == 7))
                          ps5 = ps[:, 0, :n].rearrange(
                              "p (hh r wh s) -> p r hh wh s", r=2, wh=25, s=2)
                          dest5 = tm4[:, mo, r0:r0 + nr, 1:51].rearrange(
                              "p (hh r) (wh s) -> p r hh wh s", r=2, s=2)
                          srow = 2 + r0 // 2
                          srcb = tm5[:, mo, srow:srow + nr // 2, 1:26
                                     ].unsqueeze(3).broadcast_to(
                              [128, nr // 2, 25, 2])
                          for par in range(2):
                              nc.vector.tensor_tensor(
                                  dest5[:, par], ps5[:, par], srcb, ALU.add)
                  conv3x3_wino(tp4, [(0, 16)], tm4, 0, twt4, tbtr[:, 2],
                               False, 2, vpool, mask_edge=(16, MOFF["p4"]))
                  conv3x3_wino(th4, [(0, 14)], tp4, 0, twrU, tbr1, True, 2,
                               vpool)
                  out_head(th4, 14 * 50, 2, 0)

              # ================= L3 =================
              with (tc.tile_pool(name="l3", bufs=1) as pool,
                  tc.tile_pool(name="v3", bufs=2) as vpool,
                  tc.tile_pool(name="c3chunk", bufs=3) as c3pool):
                  twl3 = pool.tile([128, 4, 2, 128], BF16, tag="wl3")
                  twt3 = pool.tile([128, 2, 2, 3, 4, 128], BF16, tag="wt3")
                  tp3 = pool.tile([128, 2, 16, 102], BF16, tag="p3")
                  th3 = pool.tile([128, 2, 14, 100], BF16, tag="h3")
                  for kh in range(4):
                      nc.sync.dma_start(out=twl3[:, kh], in_=wlat_d[1][kh])
                  for ih in range(2):
                      nc.sync.dma_start(out=twt3[:, ih], in_=wtrU_d[1][ih])
                  zero_cols(tm3, 32, [0, 101])
                  zero_cols(tp3, 16, [0, 101])

                  for ci in range(8):
                      r0 = ci * 4
                      if ci % 2 == 0:
                          tc3 = c3pool.tile([128, 4, 8, 100], BF16, tag="c3")
                          for kh in range(4):
                              nc.sync.dma_start(
                                  out=tc3[:, kh],
                                  in_=c_d[1][kh, :, r0:r0 + 8, :])
                      for mo in range(2):
                          ps = pspool.tile([128, 4, 512], F32, tag="ps")
                          for kh in range(4):
                              rhs = tc3[:, kh, (ci % 2) * 4:(ci % 2) * 4 + 4,
                                        :].rearrange("p r x -> p (r x)")
                              nc.tensor.matmul(ps[:, 0, :400],
                                               twl3[:, kh, mo],
                                               rhs, start=(kh == 0),
                                               stop=(kh == 3))
                          ps5 = ps[:, 0, :400].rearrange(
                              "p (hh r wh s) -> p r hh wh s", r=2, wh=50, s=2)
                          dest5 = tm3[:, mo, r0:r0 + 4, 1:101].rearrange(
                              "p (hh r) (wh s) -> p r hh wh s", r=2, s=2)
                          srow = 1 + r0 // 2
                          srcb = tm4[:, mo, srow:srow + 2, 1:51].unsqueeze(
                              3).broadcast_to([128, 2, 50, 2])
                          for par in range(2):
                              nc.vector.tensor_tensor(
                                  dest5[:, par], ps5[:, par], srcb, ALU.add)
                  for s in range(2):
                      conv3x3_wino(tp3, [(0, 8), (8, 8)], tm3,
                                   14 * s, twt3, tbtr[:, 1], False, 1, vpool)
                      mask_rows(tp3, MOFF["p3"] + 14 * s, [0, 15], 1, 101)
                      conv3x3_wino(th3, [(0, 8), (8, 6)], tp3, 0,
                                   twrU, tbr1, True, 1, vpool)
                      out_head(th3, 14 * 100, 1, s * 1400)

              # ================= L2 =================
              with (tc.tile_pool(name="l2", bufs=1) as pool,
                    tc.tile_pool(name="v2", bufs=2) as vpool):
                  twl2 = pool.tile([128, 2, 2, 128], BF16, tag="wl2")
                  twt2 = pool.tile([128, 2, 2, 3, 4, 128], BF16, tag="wt2")
                  tm2 = pool.tile([128, 2, 18, 202], BF16, tag="m2")
                  tp2 = pool.tile([128, 2, 16, 202], BF16, tag="p2")
                  th2 = pool.tile([128, 2, 14, 200], BF16, tag="h2")
                  for kh in range(2):
                      nc.sync.dma_start(out=twl2[:, kh], in_=wlat_d[0][kh])
                      nc.sync.dma_start(out=twt2[:, kh], in_=wtrU_d[0][kh])
                  zero_cols(tm2, 18, [0, 201])
                  zero_cols(tp2, 16, [0, 201])
                  with tc.tile_pool(name="c2chunk", bufs=6) as c2pool:
                      for s in range(4):
                          for ci in range(9):
                              r0 = ci * 2
                              tcc = c2pool.tile([128, 2, 2, 200], BF16,
                                                tag="c2")
                              for kh in range(2):
                                  nc.sync.dma_start(
                                      out=tcc[:, kh],
                                      in_=c_d[0][kh, :,
                                                 14 * s + r0:14 * s + r0 + 2,
                                                 :])
                              for mo in range(2):
                                  ps = pspool.tile([128, 4, 512], F32,
                                                   tag="ps")
                                  for kh in range(2):
                                      rhs = tcc[:, kh].rearrange(
                                          "p r x -> p (r x)")
                                      nc.tensor.matmul(
                                          ps[:, 0, :400], twl2[:, kh, mo],
                                          rhs,
                                          start=(kh == 0), stop=(kh == 1))
                                  ps4 = ps[:, 0, :400].rearrange(
                                      "p (r wh s) -> p r wh s",
                                      r=2, s=2)
                                  dest4 = tm2[:, mo, r0:r0 + 2, 1:201
                                              ].rearrange(
                                      "p r (wh s) -> p r wh s", s=2)
                                  srow = 1 + (14 * s + r0) // 2
                                  srcb = tm3[:, mo, srow, 1:101
                                             ].unsqueeze(1).unsqueeze(3)\
                                      .broadcast_to([128, 2, 100, 2])
                                  nc.vector.tensor_tensor(
                                      dest4, ps4, srcb, ALU.add)
                          conv3x3_wino(tp2, [(0, 4), (4, 4), (8, 4), (12, 4)],
                                       tm2, 0, twt2, tbtr[:, 0], False, 0,
                                       vpool)
                          mask_rows(tp2, MOFF["p2"] + 14 * s, [0, 15], 1, 201)
                          conv3x3_wino(th2, [(0, 4), (4, 4), (8, 3), (11, 3)],
                                       tp2, 0, twrU, tbr1, True, 0, vpool)
                          out_head(th2, 14 * 200, 0, s * 2800)

    nc.compile()
    return nc


def _prep_inputs(c2, c3, c4, c5, lat_w, tr_w, rpn_w1, rpn_b1, rpn_w2,
                 rpn_b2, lat_b, tr_b):
    import ml_dtypes
    bf16 = ml_dtypes.bfloat16
    cs = [c2, c3, c4, c5]
    base = {}

    def conv_w(w):
        # [O=256, I=256, 3, 3] -> [ih, 128k, mo, dy, dx, 128m]
        return np.ascontiguousarray(
            w.reshape(2, 128, 2, 128, 3, 3).transpose(2, 3, 0, 4, 5, 1)
        ).astype(bf16)

    G = np.array([[1, 0, 0], [0.5, 0.5, 0.5], [0.5, -0.5, 0.5], [0, 0, 1]],
                 np.float32)

    def conv_wU(w):
        # Winograd F(2,3) width: U[o,i,dy,t] = sum_dx G[t,dx] w[o,i,dy,dx]
        U = np.einsum("oiyx,tx->oiyt", w.astype(np.float32), G)
        return np.ascontiguousarray(
            U.reshape(2, 128, 2, 128, 3, 4).transpose(2, 3, 0, 4, 5, 1)
        ).astype(bf16)

    for i in range(4):
        base[f"wlat{i}"] = np.ascontiguousarray(
            lat_w[i].reshape(2, 128, KH[i], 128).transpose(2, 3, 0, 1)
        ).astype(bf16)
    for i in range(3):
        base[f"wtrU{i}"] = conv_wU(tr_w[i])
    base["wtr3"] = conv_w(tr_w[3])
    base["wrpn1"] = conv_w(rpn_w1)
    base["wrpnU"] = conv_wU(rpn_w1)
    w2 = np.zeros((2, 128, 16), np.float32)
    w2[:, :, :15] = rpn_w2.reshape(15, 2, 128).transpose(1, 2, 0)
    base["wrpn2"] = w2.astype(bf16)
    base["btr"] = np.ascontiguousarray(
        np.stack([b.reshape(2, 128).T for b in tr_b]).transpose(0, 1, 2))
    base["brpn1"] = np.ascontiguousarray(rpn_b1.reshape(2, 128).T)
    b2 = np.zeros((16, 1), np.float32)
    b2[:15, 0] = rpn_b2
    base["brpn2"] = b2
    base["zeros"] = np.zeros((128, 512), bf16)
    for b in lat_b:
        if np.abs(b).max() != 0:
            raise NotImplementedError("nonzero lateral bias not supported")

    in_maps = []
    starts = [S2, S3, S4, S5]
    m_off = [2, 2, 2, 3]
    for b in range(2):
        for q in range(4):
            m = dict(base)
            for i in range(4):
                r0 = starts[i][q] - m_off[i]
                rows = M_ROWS[i]
                sl = np.zeros((CIN[i], rows, CW[i]), np.float32)
                lo, hi = max(0, r0), min(H[i], r0 + rows)
                if hi > lo:
                    sl[:, lo - r0:hi - r0, :W[i]] = cs[i][b, :, lo:hi, :]
                m[f"c{i}"] = np.ascontiguousarray(
                    sl.reshape(KH[i], 128, rows, CW[i])).astype(bf16)
            mk = np.zeros(244, np.float32)
            spans = [(S2[q] - 2, 200, 60), (S2[q] - 1, 200, 58),
                     (S3[q] - 2, 100, 32), (S3[q] - 1, 100, 30),
                     (S4[q] - 2, 50, 18), (S4[q] - 1, 50, 16),
                     (S5[q] - 3, 25, 13), (S5[q] - 2, 25, 11),
                     (S6[q] - 1, 13, 6)]
            off = 0
            for a0, hh, ln in spans:
                for j in range(ln):
                    mk[off + j] = 1.0 if 0 <= a0 + j < hh else 0.0
                off += ln
            m["masks"] = np.broadcast_to(mk, (128, 244)).astype(bf16)
            in_maps.append(m)
    return in_maps


def _assemble(results):
    out = np.zeros((2, 159882, 5), np.float32)
    lvl_off = [0, 120000, 150000, 157500, 159375]
    starts = [S2, S3, S4, S5, S6]
    for b in range(2):
        for q in range(4):
            r = results[b * 4 + q]
            for lv in range(5):
                v, w = VALID[lv][q], W[lv]
                # device layout is [ch=15, pix]; transpose to pixel-major
                o = np.ascontiguousarray(
                    r[f"o{lv}"].reshape(15, OWN[lv] * w).T
                ).reshape(OWN[lv] * w * 3, 5)
                a0 = lvl_off[lv] + starts[lv][q] * w * 3
                out[b, a0:a0 + v * w * 3] = o[:v * w * 3]
    return out


def kernel(**inputs):
    import os
    from concourse.bass_utils import run_bass_kernel_spmd
    if "nc" not in _CACHED:
        _CACHED["nc"] = _build(int(os.environ.get("K_REPS", "1")))
    nc = _CACHED["nc"]
    in_maps = _prep_inputs(
        np.asarray(inputs["c2"], np.float32),
        np.asarray(inputs["c3"], np.float32),
        np.asarray(inputs["c4"], np.float32),
        np.asarray(inputs["c5"], np.float32),
        [np.asarray(inputs[f"lat_w{i}"], np.float32) for i in range(4)],
        [np.asarray(inputs[f"tr_w{i}"], np.float32) for i in range(4)],
        np.asarray(inputs["rpn_w1"], np.float32),
        np.asarray(inputs["rpn_b1"], np.float32),
        np.asarray(inputs["rpn_w2"], np.float32),
        np.asarray(inputs["rpn_b2"], np.float32),
        [np.asarray(inputs[f"lat_b{i}"], np.float32) for i in range(4)],
        [np.asarray(inputs[f"tr_b{i}"], np.float32) for i in range(4)],
    )
    res = run_bass_kernel_spmd(nc, in_maps, list(range(N_CORES)))
    return _assemble(res.results)


# revision 10
# speedup vs baseline: 1.0054x; 1.0054x over previous
"""FPN + RPN detector kernel for Trainium2, 8-core SPMD.

Sharding: core = (image b in {0,1}) x (row-quarter q in {0..3}). Host slices
inputs with halos (zero-padded), every core runs the same program on its
slice, host crops/reassembles. No cross-core communication.

All convs are bf16 matmuls (full PE rate at any N, fast weight load),
channels on partitions in two halves of 128, PSUM f32 accumulation over
taps/halves; biases and outputs stay f32.

The 3x3 convs at L2/L3/L4 (tr + rpn1) use 1-D Winograd F(2,3) along the
width: per 2 output cols, 4 transform points x 3 dy x 2 ih = 24 matmul
cycles/2px vs 36 direct (1.5x less PE work). Input transform V on DVE,
output transform A^T on DVE/ACT with fused bias. L5/L6 stay direct.
"""
import numpy as np

N_CORES = 8

# Quarter starts chosen so every upsample/subsample alignment offset is
# uniform across cores (SPMD requirement).
S2 = [0, 48, 96, 144]
S3 = [0, 24, 48, 72]
S4 = [0, 12, 24, 36]
S5 = [0, 6, 12, 18]
S6 = [0, 3, 6, 9]
H = [200, 100, 50, 25, 13]
W = [200, 100, 50, 25, 13]
OWN = [56, 28, 14, 7, 4]
VALID = [
    [48, 48, 48, 56],
    [24, 24, 24, 28],
    [12, 12, 12, 14],
    [6, 6, 6, 7],
    [3, 3, 3, 4],
]
M_ROWS = [60, 32, 18, 13]     # m2..m5 region rows (m_l from S_l-2, m5 S5-3)
CIN = [256, 512, 1024, 2048]
KH = [2, 4, 8, 16]
CW = [200, 100, 50, 26]       # c-slice widths (c5 x-padded to 26)
CONVW = [200, 100, 50, 26, 14]  # conv window x-counts (even)

_CACHED = {}


def _build(reps=1):
    import concourse.bacc as bacc
    import concourse.mybir as mybir
    from concourse.tile import TileContext

    F32, BF16 = mybir.dt.float32, mybir.dt.bfloat16
    AF = mybir.ActivationFunctionType
    ALU = mybir.AluOpType

    nc = bacc.Bacc("TRN2", target_bir_lowering=False, debug=False,
                   num_devices=N_CORES)

    c_d = [nc.dram_tensor(f"c{i}", [KH[i], 128, M_ROWS[i], CW[i]], BF16,
                          kind="ExternalInput") for i in range(4)]
    wlat_d = [nc.dram_tensor(f"wlat{i}", [KH[i], 128, 2, 128], BF16,
                             kind="ExternalInput") for i in range(4)]
    # direct 3x3 weights (L5 tr conv + L5/L6 rpn conv)
    wtr3_d = nc.dram_tensor("wtr3", [2, 128, 2, 3, 3, 128], BF16,
                            kind="ExternalInput")
    wrpn1_d = nc.dram_tensor("wrpn1", [2, 128, 2, 3, 3, 128], BF16,
                             kind="ExternalInput")
    # Winograd F(2,3) width-transformed weights (t axis of 4)
    wtrU_d = [nc.dram_tensor(f"wtrU{i}", [2, 128, 2, 3, 4, 128], BF16,
                             kind="ExternalInput") for i in range(3)]
    wrpnU_d = nc.dram_tensor("wrpnU", [2, 128, 2, 3, 4, 128], BF16,
                             kind="ExternalInput")
    wrpn2_d = nc.dram_tensor("wrpn2", [2, 128, 16], BF16,
                             kind="ExternalInput")
    btr_d = nc.dram_tensor("btr", [4, 128, 2], F32, kind="ExternalInput")
    brpn1_d = nc.dram_tensor("brpn1", [128, 2], F32, kind="ExternalInput")
    brpn2_d = nc.dram_tensor("brpn2", [16, 1], F32, kind="ExternalInput")
    zeros_d = nc.dram_tensor("zeros", [128, 512], BF16, kind="ExternalInput")
    masks_d = nc.dram_tensor("masks", [128, 244], BF16, kind="ExternalInput")
    MOFF = {}
    _off = 0
    for name, n in [("m2", 60), ("p2", 58), ("m3", 32), ("p3", 30),
                    ("m4", 18), ("p4", 16), ("m5", 13), ("p5", 11),
                    ("p6", 6)]:
        MOFF[name] = _off
        _off += n

    o_d = [nc.dram_tensor(f"o{i}", [OWN[i] * W[i] * 3 * 5], F32,
                          kind="ExternalOutput") for i in range(5)]

    with TileContext(nc, pool_alloc_mode="queue") as tc:
        with (
            tc.tile_pool(name="const", bufs=1) as cpool,
            tc.tile_pool(name="ps", bufs=2, space="PSUM") as pspool,
            tc.tile_pool(name="stg", bufs=1) as stgpool,
            tc.tile_pool(name="escr", bufs=2) as epool,
        ):
            tz = cpool.tile([128, 512], BF16, tag="zeros")
            tmask = cpool.tile([128, 244], BF16, tag="masks")
            twrU = cpool.tile([128, 2, 2, 3, 4, 128], BF16, tag="wrpnU")
            twr2 = cpool.tile([128, 2, 16], BF16, tag="wrpn2")
            tbtr = cpool.tile([128, 4, 2], F32, tag="btr")
            tbr1 = cpool.tile([128, 2], F32, tag="brpn1")
            tbr2 = cpool.tile([16, 1], F32, tag="brpn2")
            nc.gpsimd.dma_start(out=tz[:], in_=zeros_d[:])
            nc.gpsimd.dma_start(out=tmask[:], in_=masks_d[:])
            nc.gpsimd.dma_start(out=tbtr[:],
                                in_=btr_d.rearrange("l p t -> p l t"))
            nc.gpsimd.dma_start(out=tbr1[:], in_=brpn1_d[:])
            nc.gpsimd.dma_start(out=tbr2[:], in_=brpn2_d[:])
            for ih in range(2):
                nc.sync.dma_start(out=twr2[:, ih], in_=wrpn2_d[ih])
                for mo in range(2):
                    nc.sync.dma_start(out=twrU[:, ih, mo],
                                      in_=wrpnU_d[ih, :, mo])

            tm3 = cpool.tile([128, 2, 32, 102], BF16, tag="m3")
            tm4 = cpool.tile([128, 2, 18, 52], BF16, tag="m4")
            tm5 = cpool.tile([128, 2, 13, 28], BF16, tag="m5")

            def zero_cols(t, nrows, cols):
                for ih in range(2):
                    for c0 in cols:
                        nc.vector.tensor_copy(
                            t[:, ih, :, c0].squeeze(), tz[:, :nrows])

            def mask_rows(t, moff, rows, i0, i1):
                for ih in range(2):
                    for r in rows:
                        mk = tmask[:, moff + r:moff + r + 1].broadcast_to(
                            [128, i1 - i0])
                        nc.vector.tensor_tensor(
                            t[:, ih, r, i0:i1], t[:, ih, r, i0:i1], mk,
                            ALU.mult)

            def conv3x3(dst_tile, groups, src_tile, src_row_of, lhsT,
                        bias, relu, lvl, mask_edge=None):
                """Direct 3x3 conv (L5/L6): dst row j reads src rows
                j+src_row_of+dy. relu=True -> unpadded h tile, else padded
                p tile (interior cols 1..W). groups = [(j0, nrows)]."""
                wv, wl = CONVW[lvl], W[lvl]
                for (j0, nr) in groups:
                    for mo in range(2):
                        ps = pspool.tile([128, 4, 512], F32, tag="ps")
                        n = nr * wv
                        first = True
                        for ih in range(2):
                            for dy in range(3):
                                for dx in range(3):
                                    rhs = src_tile[
                                        :, ih,
                                        j0 + src_row_of + dy:
                                        j0 + src_row_of + dy + nr,
                                        dx:dx + wv]
                                    nc.tensor.matmul(
                                        ps[:, 0, :n], lhsT[:, ih, mo, dy, dx],
                                        rhs, start=first,
                                        stop=(ih == 1 and dy == 2 and dx == 2))
                                    first = False
                        psv = ps[:, 0, :n].rearrange("p (r x) -> p r x", x=wv)
                        if relu:
                            dst = dst_tile[:, mo, j0:j0 + nr, :wl]
                            nc.scalar.activation(dst, psv[:, :, :wl], AF.Relu,
                                                 bias=bias[:, mo:mo + 1])
                        else:
                            dst = dst_tile[:, mo, j0:j0 + nr, 1:1 + wl]
                            nc.scalar.activation(dst, psv[:, :, :wl],
                                                 AF.Identity,
                                                 bias=bias[:, mo:mo + 1])
                if mask_edge is not None:
                    nrt, moff = mask_edge
                    mask_rows(dst_tile, moff, [0, 1, nrt - 2, nrt - 1],
                              1, 1 + wl)

            def conv3x3_wino(dst_tile, groups, src_tile, src_row_of, Ut,
                             bias, relu, lvl, vpool, mask_edge=None):
                """1-D Winograd F(2,3) width conv. Same contract as conv3x3.

                V layout [128, ih, 4t, vr, Wg]; per group (j0, nr) and mo,
                P_t accumulates in PSUM bank t; A^T drain:
                  even(2g+1) = P0+P1+P2+b ; odd(2g+2) = P1-P2-P3+b.
                """
                wl = W[lvl]
                wg = wl // 2
                r0 = min(j0 for j0, _ in groups)
                r1 = max(j0 + nr for j0, nr in groups)
                vr = r1 - r0 + 2
                tv = vpool.tile([128, 2, 4, vr, wg], BF16, tag="v")
                sview = src_tile[
                    :, :, src_row_of + r0:src_row_of + r1 + 2,
                    :2 * wg + 2].rearrange("p i r (g s) -> p i r g s", s=2)
                for ih in range(2):
                    d0 = sview[:, ih, :, 0:wg, 0]
                    d1 = sview[:, ih, :, 0:wg, 1]
                    d2 = sview[:, ih, :, 1:wg + 1, 0]
                    d3 = sview[:, ih, :, 1:wg + 1, 1]
                    # split V work across DVE and the otherwise-idle GpSimd
                    nc.vector.tensor_tensor(tv[:, ih, 0], d0, d2,
                                            ALU.subtract)
                    nc.gpsimd.tensor_tensor(tv[:, ih, 1], d1, d2, ALU.add)
                    nc.vector.tensor_tensor(tv[:, ih, 2], d2, d1,
                                            ALU.subtract)
                    nc.gpsimd.tensor_tensor(tv[:, ih, 3], d1, d3,
                                            ALU.subtract)
                for (j0, nr) in groups:
                    n = nr * wg
                    for mo in range(2):
                        ps = pspool.tile([128, 4, 512], F32, tag="ps")
                        for t in range(4):
                            for ih in range(2):
                                for dy in range(3):
                                    rhs = tv[:, ih, t,
                                             j0 - r0 + dy:j0 - r0 + dy + nr]
                                    nc.tensor.matmul(
                                        ps[:, t, :n], Ut[:, ih, mo, dy, t],
                                        rhs, start=(ih == 0 and dy == 0),
                                        stop=(ih == 1 and dy == 2))
                        # drains: C1 = P1 + b; even = P0+C1+P2; odd = C1-P3-P2
                        tc1 = epool.tile([128, 512], F32, tag="c1")
                        te = epool.tile([128, 512], F32, tag="e")
                        to = epool.tile([128, 512], F32, tag="o")
                        nc.scalar.activation(tc1[:, :n], ps[:, 1, :n],
                                             AF.Identity,
                                             bias=bias[:, mo:mo + 1])
                        if relu:
                            dv = dst_tile[:, mo, j0:j0 + nr, :2 * wg
                                          ].rearrange(
                                "p r (g s) -> p r g s", s=2)
                            d_ev = dv[:, :, :, 0]
                            d_od = dv[:, :, :, 1]
                        else:
                            dv = dst_tile[:, mo, j0:j0 + nr, :2 * wg + 2
                                          ].rearrange(
                                "p r (g s) -> p r g s", s=2)
                            d_ev = dv[:, :, 0:wg, 1]
                            d_od = dv[:, :, 1:wg + 1, 0]
                        c1v = tc1[:, :n].rearrange("p (r g) -> p r g", g=wg)
                        ev = te[:, :n].rearrange("p (r g) -> p r g", g=wg)
                        ov = to[:, :n].rearrange("p (r g) -> p r g", g=wg)
                        p0 = ps[:, 0, :n].rearrange("p (r g) -> p r g", g=wg)
                        p2 = ps[:, 2, :n].rearrange("p (r g) -> p r g", g=wg)
                        p3 = ps[:, 3, :n].rearrange("p (r g) -> p r g", g=wg)
                        nc.vector.tensor_tensor(ev, p0, c1v, ALU.add)
                        nc.vector.scalar_tensor_tensor(
                            ov, p3, -1.0, c1v, ALU.mult, ALU.add)
                        if relu:
                            tf = epool.tile([128, 512], F32, tag="f")
                            tf2 = epool.tile([128, 512], F32, tag="f")
                            fv = tf[:, :n].rearrange("p (r g) -> p r g", g=wg)
                            f2v = tf2[:, :n].rearrange("p (r g) -> p r g",
                                                       g=wg)
                            nc.vector.tensor_tensor(fv, ev, p2, ALU.add)
                            nc.vector.scalar_tensor_tensor(
                                f2v, p2, -1.0, ov, ALU.mult, ALU.add)
                            nc.scalar.activation(d_ev, fv, AF.Relu, bias=0.0)
                            nc.scalar.activation(d_od, f2v, AF.Relu, bias=0.0)
                        else:
                            nc.vector.tensor_tensor(d_ev, ev, p2, ALU.add)
                            nc.vector.scalar_tensor_tensor(
                                d_od, p2, -1.0, ov, ALU.mult, ALU.add)
                if mask_edge is not None:
                    nrt, moff = mask_edge
                    mask_rows(dst_tile, moff, [0, 1, nrt - 2, nrt - 1],
                              1, 1 + wl)

            def out_head(h_tile, npix, lvl, px0):
                # o = W2^T h, channel-major: twr2 stationary (16-col weight
                # loads), h streamed in 512-px chunks. Output layout in DRAM
                # is [ch=15, pix] per level; host transposes in _assemble.
                hflat = h_tile.rearrange("p t r x -> p t (r x)")
                tstage = stgpool.tile([16, 2800], F32, tag="ostage")
                for g in range((npix + 511) // 512):
                    p0 = g * 512
                    n = min(512, npix - p0)
                    pso = pspool.tile([128, 4, 512], F32, tag="ps")
                    for ih in range(2):
                        nc.tensor.matmul(
                            pso[:16, 0, :n], twr2[:, ih],
                            hflat[:, ih, p0:p0 + n],
                            start=(ih == 0), stop=(ih == 1))
                    nc.scalar.activation(tstage[:, p0:p0 + n],
                                         pso[:16, 0, :n],
                                         AF.Identity, bias=tbr2[:, 0:1])
                dst = o_d[lvl].rearrange("(c pix) -> c pix", c=15)[
                    :, px0:px0 + npix]
                nc.sync.dma_start(out=dst, in_=tstage[:15, :npix])

            # ================= L5 (+L6) =================
            for _rep in range(reps):
              with tc.tile_pool(name="l5", bufs=1) as pool:
                  tc5 = pool.tile([128, 16, 13, 26], BF16, tag="c5")
                  twl5 = pool.tile([128, 16, 2, 128], BF16, tag="wl5")
                  twt5 = pool.tile([128, 2, 2, 3, 3, 128], BF16, tag="wt5")
                  twr1 = pool.tile([128, 2, 2, 3, 3, 128], BF16, tag="wr1")
                  tp5 = pool.tile([128, 2, 11, 28], BF16, tag="p5")
                  tp6 = pool.tile([128, 2, 6, 16], BF16, tag="p6")
                  th5 = pool.tile([128, 2, 7, 25], BF16, tag="h5")
                  th6 = pool.tile([128, 2, 4, 13], BF16, tag="h6")
                  for kh in range(16):
                      nc.sync.dma_start(out=tc5[:, kh], in_=c_d[3][kh])
                      nc.sync.dma_start(out=twl5[:, kh], in_=wlat_d[3][kh])
                  for ih in range(2):
                      nc.sync.dma_start(out=twt5[:, ih], in_=wtr3_d[ih])
                      for mo in range(2):
                          nc.sync.dma_start(out=twr1[:, ih, mo],
                                            in_=wrpn1_d[ih, :, mo])
                  zero_cols(tm5, 13, [0, 26, 27])
                  zero_cols(tp5, 11, [0, 26, 27])
                  zero_cols(tp6, 6, [0, 14, 15])

                  for mo in range(2):
                      ps = pspool.tile([128, 4, 512], F32, tag="ps")
                      n = 13 * 26
                      for kh in range(16):
                          nc.tensor.matmul(
                              ps[:, 0, :n], twl5[:, kh, mo],
                              tc5[:, kh].rearrange("p r x -> p (r x)"),
                              start=(kh == 0), stop=(kh == 15))
                      psv = ps[:, 0, :n].rearrange("p (r x) -> p r x", x=26)
                      nc.scalar.activation(tm5[:, mo, :, 1:26], psv[:, :, :25],
                                           AF.Copy, bias=0.0)

                  conv3x3(tp5, [(0, 11)], tm5, 0, twt5, tbtr[:, 3], False, 3,
                          mask_edge=(11, MOFF["p5"]))
                  # p6 = p5[::2, ::2]: row j <- p5 row 2j, col x <- p5 col 1+2x
                  for ih in range(2):
                      for j in range(6):
                          src = tp5[:, ih, 2 * j, 1:27].rearrange(
                              "p (x s) -> p x s", s=2)[:, :, 0].squeeze()
                          nc.vector.tensor_copy(tp6[:, ih, j, 1:14], src)
                  conv3x3(th5, [(0, 7)], tp5, 1, twr1, tbr1, True, 3)
                  conv3x3(th6, [(0, 4)], tp6, 0, twr1, tbr1, True, 4)
                  out_head(th5, 7 * 25, 3, 0)
                  out_head(th6, 4 * 13, 4, 0)

              # ================= L4 =================
              with (tc.tile_pool(name="l4", bufs=1) as pool,
                  tc.tile_pool(name="v4", bufs=2) as vpool,
                  tc.tile_pool(name="c4chunk", bufs=3) as c4pool):
                  twl4 = pool.tile([128, 8, 2, 128], BF16, tag="wl4")
                  twt4 = pool.tile([128, 2, 2, 3, 4, 128], BF16, tag="wt4")
                  tp4 = pool.tile([128, 2, 16, 52], BF16, tag="p4")
                  th4 = pool.tile([128, 2, 14, 50], BF16, tag="h4")
                  for kh in range(8):
                      nc.sync.dma_start(out=twl4[:, kh], in_=wlat_d[2][kh])
                  for ih in range(2):
                      nc.sync.dma_start(out=twt4[:, ih], in_=wtrU_d[2][ih])
                  zero_cols(tm4, 18, [0, 51])
                  zero_cols(tp4, 16, [0, 51])

                  for (r0, nr) in [(0, 6), (6, 6), (12, 6)]:
                      tc4 = c4pool.tile([128, 8, 6, 50], BF16, tag="c4")
                      for kh in range(8):
                          nc.sync.dma_start(out=tc4[:, kh],
                                            in_=c_d[2][kh, :, r0:r0 + nr, :])
                      for mo in range(2):
                          ps = pspool.tile([128, 4, 512], F32, tag="ps")
                          n = nr * 50
                          for kh in range(8):
                              rhs = tc4[:, kh].rearrange("p r x -> p (r x)")
                              nc.tensor.matmul(ps[:, 0, :n], twl4[:, kh, mo],
                                               rhs,
                                               start=(kh == 0), stop=(kh == 7))
                          ps5 = ps[:, 0, :n].rearrange(
                              "p (hh r wh s) -> p r hh wh s", r=2, wh=25, s=2)
                          dest5 = tm4[:, mo, r0:r0 + nr, 1:51].rearrange(
                              "p (hh r) (wh s) -> p r hh wh s", r=2, s=2)
                          srow = 2 + r0 // 2
                          srcb = tm5[:, mo, srow:srow + nr // 2, 1:26
                                     ].unsqueeze(3).broadcast_to(
                              [128, nr // 2, 25, 2])
                          for par in range(2):
                              nc.vector.tensor_tensor(
                                  dest5[:, par], ps5[:, par], srcb, ALU.add)
                  conv3x3_wino(tp4, [(0, 16)], tm4, 0, twt4, tbtr[:, 2],
                               False, 2, vpool, mask_edge=(16, MOFF["p4"]))
                  conv3x3_wino(th4, [(0, 14)], tp4, 0, twrU, tbr1, True, 2,
                               vpool)
                  out_head(th4, 14 * 50, 2, 0)

              # ================= L3 =================
              with (tc.tile_pool(name="l3", bufs=1) as pool,
                  tc.tile_pool(name="v3", bufs=2) as vpool,
                  tc.tile_pool(name="p3pool", bufs=2) as p3pool,
                  tc.tile_pool(name="c3chunk", bufs=4) as c3pool):
                  twl3 = pool.tile([128, 4, 2, 128], BF16, tag="wl3")
                  twt3 = pool.tile([128, 2, 2, 3, 4, 128], BF16, tag="wt3")
                  th3 = pool.tile([128, 2, 14, 100], BF16, tag="h3")
                  for kh in range(4):
                      nc.sync.dma_start(out=twl3[:, kh], in_=wlat_d[1][kh])
                  for ih in range(2):
                      nc.sync.dma_start(out=twt3[:, ih], in_=wtrU_d[1][ih])
                  zero_cols(tm3, 32, [0, 101])

                  for ci in range(8):
                      r0 = ci * 4
                      if ci % 2 == 0:
                          tc3 = c3pool.tile([128, 4, 8, 100], BF16, tag="c3")
                          for kh in range(4):
                              nc.sync.dma_start(
                                  out=tc3[:, kh],
                                  in_=c_d[1][kh, :, r0:r0 + 8, :])
                      for mo in range(2):
                          ps = pspool.tile([128, 4, 512], F32, tag="ps")
                          for kh in range(4):
                              rhs = tc3[:, kh, (ci % 2) * 4:(ci % 2) * 4 + 4,
                                        :].rearrange("p r x -> p (r x)")
                              nc.tensor.matmul(ps[:, 0, :400],
                                               twl3[:, kh, mo],
                                               rhs, start=(kh == 0),
                                               stop=(kh == 3))
                          ps5 = ps[:, 0, :400].rearrange(
                              "p (hh r wh s) -> p r hh wh s", r=2, wh=50, s=2)
                          dest5 = tm3[:, mo, r0:r0 + 4, 1:101].rearrange(
                              "p (hh r) (wh s) -> p r hh wh s", r=2, s=2)
                          srow = 1 + r0 // 2
                          srcb = tm4[:, mo, srow:srow + 2, 1:51].unsqueeze(
                              3).broadcast_to([128, 2, 50, 2])
                          for par in range(2):
                              nc.vector.tensor_tensor(
                                  dest5[:, par], ps5[:, par], srcb, ALU.add)
                  # pipeline: both tr convs first, then rpn+head per s, so
                  # V-transforms for stage k compute while PE runs stage k-1
                  tp3s = []
                  for s in range(2):
                      tp3 = p3pool.tile([128, 2, 16, 102], BF16, tag="p3")
                      zero_cols(tp3, 16, [0, 101])
                      conv3x3_wino(tp3, [(0, 8), (8, 8)], tm3,
                                   14 * s, twt3, tbtr[:, 1], False, 1, vpool)
                      mask_rows(tp3, MOFF["p3"] + 14 * s, [0, 15], 1, 101)
                      tp3s.append(tp3)
                  for s in range(2):
                      conv3x3_wino(th3, [(0, 8), (8, 6)], tp3s[s], 0,
                                   twrU, tbr1, True, 1, vpool)
                      out_head(th3, 14 * 100, 1, s * 1400)

              # ================= L2 =================
              with (tc.tile_pool(name="l2", bufs=1) as pool,
                    tc.tile_pool(name="p2pool", bufs=2) as p2pool,
                    tc.tile_pool(name="v2", bufs=2) as vpool):
                  twl2 = pool.tile([128, 2, 2, 128], BF16, tag="wl2")
                  twt2 = pool.tile([128, 2, 2, 3, 4, 128], BF16, tag="wt2")
                  tm2 = pool.tile([128, 2, 18, 202], BF16, tag="m2")
                  th2 = pool.tile([128, 2, 14, 200], BF16, tag="h2")
                  for kh in range(2):
                      nc.sync.dma_start(out=twl2[:, kh], in_=wlat_d[0][kh])
                      nc.sync.dma_start(out=twt2[:, kh], in_=wtrU_d[0][kh])
                  zero_cols(tm2, 18, [0, 201])

                  def rpn_head2(s, tp2):
                      conv3x3_wino(th2, [(0, 4), (4, 4)],
                                   tp2, 0, twrU, tbr1, True, 0, vpool)
                      conv3x3_wino(th2, [(8, 3), (11, 3)],
                                   tp2, 0, twrU, tbr1, True, 0, vpool)
                      out_head(th2, 14 * 200, 0, s * 2800)

                  # 2-stage software pipeline: rpn/head lag the tr conv by
                  # two s-chunks so V-transforms hide under PE matmul work.
                  with tc.tile_pool(name="c2chunk", bufs=3) as c2pool:
                      pend = []
                      for s in range(4):
                          for ci in range(9):
                              r0 = ci * 2
                              tcc = c2pool.tile([128, 2, 2, 200], BF16,
                                                tag="c2")
                              for kh in range(2):
                                  nc.sync.dma_start(
                                      out=tcc[:, kh],
                                      in_=c_d[0][kh, :,
                                                 14 * s + r0:14 * s + r0 + 2,
                                                 :])
                              for mo in range(2):
                                  ps = pspool.tile([128, 4, 512], F32,
                                                   tag="ps")
                                  for kh in range(2):
                                      rhs = tcc[:, kh].rearrange(
                                          "p r x -> p (r x)")
                                      nc.tensor.matmul(
                                          ps[:, 0, :400], twl2[:, kh, mo],
                                          rhs,
                                          start=(kh == 0), stop=(kh == 1))
                                  ps4 = ps[:, 0, :400].rearrange(
                                      "p (r wh s) -> p r wh s",
                                      r=2, s=2)
                                  dest4 = tm2[:, mo, r0:r0 + 2, 1:201
                                              ].rearrange(
                                      "p r (wh s) -> p r wh s", s=2)
                                  srow = 1 + (14 * s + r0) // 2
                                  srcb = tm3[:, mo, srow, 1:101
                                             ].unsqueeze(1).unsqueeze(3)\
                                      .broadcast_to([128, 2, 100, 2])
                                  nc.vector.tensor_tensor(
                                      dest4, ps4, srcb, ALU.add)
                          if len(pend) >= 2:
                              rpn_head2(*pend.pop(0))
                          tp2 = p2pool.tile([128, 2, 16, 202], BF16,
                                            tag="p2")
                          zero_cols(tp2, 16, [0, 201])
                          conv3x3_wino(tp2, [(0, 4), (4, 4)],
                                       tm2, 0, twt2, tbtr[:, 0], False, 0,
                                       vpool)
                          conv3x3_wino(tp2, [(8, 4), (12, 4)],
                                       tm2, 0, twt2, tbtr[:, 0], False, 0,
                                       vpool)
                          mask_rows(tp2, MOFF["p2"] + 14 * s, [0, 15], 1, 201)
                          pend.append((s, tp2))
                      for item in pend:
                          rpn_head2(*item)

    nc.compile()
    return nc


def _prep_inputs(c2, c3, c4, c5, lat_w, tr_w, rpn_w1, rpn_b1, rpn_w2,
                 rpn_b2, lat_b, tr_b):
    import ml_dtypes
    bf16 = ml_dtypes.bfloat16
    cs = [c2, c3, c4, c5]
    base = {}

    def conv_w(w):
        # [O=256, I=256, 3, 3] -> [ih, 128k, mo, dy, dx, 128m]
        return np.ascontiguousarray(
            w.reshape(2, 128, 2, 128, 3, 3).transpose(2, 3, 0, 4, 5, 1)
        ).astype(bf16)

    G = np.array([[1, 0, 0], [0.5, 0.5, 0.5], [0.5, -0.5, 0.5], [0, 0, 1]],
                 np.float32)

    def conv_wU(w):
        # Winograd F(2,3) width: U[o,i,dy,t] = sum_dx G[t,dx] w[o,i,dy,dx]
        U = np.einsum("oiyx,tx->oiyt", w.astype(np.float32), G)
        return np.ascontiguousarray(
            U.reshape(2, 128, 2, 128, 3, 4).transpose(2, 3, 0, 4, 5, 1)
        ).astype(bf16)

    for i in range(4):
        base[f"wlat{i}"] = np.ascontiguousarray(
            lat_w[i].reshape(2, 128, KH[i], 128).transpose(2, 3, 0, 1)
        ).astype(bf16)
    for i in range(3):
        base[f"wtrU{i}"] = conv_wU(tr_w[i])
    base["wtr3"] = conv_w(tr_w[3])
    base["wrpn1"] = conv_w(rpn_w1)
    base["wrpnU"] = conv_wU(rpn_w1)
    w2 = np.zeros((2, 128, 16), np.float32)
    w2[:, :, :15] = rpn_w2.reshape(15, 2, 128).transpose(1, 2, 0)
    base["wrpn2"] = w2.astype(bf16)
    base["btr"] = np.ascontiguousarray(
        np.stack([b.reshape(2, 128).T for b in tr_b]).transpose(0, 1, 2))
    base["brpn1"] = np.ascontiguousarray(rpn_b1.reshape(2, 128).T)
    b2 = np.zeros((16, 1), np.float32)
    b2[:15, 0] = rpn_b2
    base["brpn2"] = b2
    base["zeros"] = np.zeros((128, 512), bf16)
    for b in lat_b:
        if np.abs(b).max() != 0:
            raise NotImplementedError("nonzero lateral bias not supported")

    in_maps = []
    starts = [S2, S3, S4, S5]
    m_off = [2, 2, 2, 3]
    for b in range(2):
        for q in range(4):
            m = dict(base)
            for i in range(4):
                r0 = starts[i][q] - m_off[i]
                rows = M_ROWS[i]
                sl = np.zeros((CIN[i], rows, CW[i]), np.float32)
                lo, hi = max(0, r0), min(H[i], r0 + rows)
                if hi > lo:
                    sl[:, lo - r0:hi - r0, :W[i]] = cs[i][b, :, lo:hi, :]
                m[f"c{i}"] = np.ascontiguousarray(
                    sl.reshape(KH[i], 128, rows, CW[i])).astype(bf16)
            mk = np.zeros(244, np.float32)
            spans = [(S2[q] - 2, 200, 60), (S2[q] - 1, 200, 58),
                     (S3[q] - 2, 100, 32), (S3[q] - 1, 100, 30),
                     (S4[q] - 2, 50, 18), (S4[q] - 1, 50, 16),
                     (S5[q] - 3, 25, 13), (S5[q] - 2, 25, 11),
                     (S6[q] - 1, 13, 6)]
            off = 0
            for a0, hh, ln in spans:
                for j in range(ln):
                    mk[off + j] = 1.0 if 0 <= a0 + j < hh else 0.0
                off += ln
            m["masks"] = np.broadcast_to(mk, (128, 244)).astype(bf16)
            in_maps.append(m)
    return in_maps


def _assemble(results):
    out = np.zeros((2, 159882, 5), np.float32)
    lvl_off = [0, 120000, 150000, 157500, 159375]
    starts = [S2, S3, S4, S5, S6]
    for b in range(2):
        for q in range(4):
            r = results[b * 4 + q]
            for lv in range(5):
                v, w = VALID[lv][q], W[lv]
                # device layout is [ch=15, pix]; transpose to pixel-major
                o = np.ascontiguousarray(
                    r[f"o{lv}"].reshape(15, OWN[lv] * w).T
                ).reshape(OWN[lv] * w * 3, 5)
                a0 = lvl_off[lv] + starts[lv][q] * w * 3
                out[b, a0:a0 + v * w * 3] = o[:v * w * 3]
    return out


def kernel(**inputs):
    import os
    from concourse.bass_utils import run_bass_kernel_spmd
    if "nc" not in _CACHED:
        _CACHED["nc"] = _build(int(os.environ.get("K_REPS", "1")))
    nc = _CACHED["nc"]
    in_maps = _prep_inputs(
        np.asarray(inputs["c2"], np.float32),
        np.asarray(inputs["c3"], np.float32),
        np.asarray(inputs["c4"], np.float32),
        np.asarray(inputs["c5"], np.float32),
        [np.asarray(inputs[f"lat_w{i}"], np.float32) for i in range(4)],
        [np.asarray(inputs[f"tr_w{i}"], np.float32) for i in range(4)],
        np.asarray(inputs["rpn_w1"], np.float32),
        np.asarray(inputs["rpn_b1"], np.float32),
        np.asarray(inputs["rpn_w2"], np.float32),
        np.asarray(inputs["rpn_b2"], np.float32),
        [np.asarray(inputs[f"lat_b{i}"], np.float32) for i in range(4)],
        [np.asarray(inputs[f"tr_b{i}"], np.float32) for i in range(4)],
    )
    res = run_bass_kernel_spmd(nc, in_maps, list(range(N_CORES)))
    return _assemble(res.results)


# revision 15
# speedup vs baseline: 1.1511x; 1.1449x over previous
"""FPN + RPN detector kernel for Trainium2, 8-core SPMD.

Sharding: core = (image b in {0,1}) x (row-quarter q in {0..3}). Host slices
inputs with halos (zero-padded), every core runs the same program on its
slice, host crops/reassembles. No cross-core communication.

All convs are bf16 matmuls (full PE rate at any N, fast weight load),
channels on partitions in two halves of 128, PSUM f32 accumulation over
taps/halves; biases and outputs stay f32.

The 3x3 convs at L2/L3/L4 (tr + rpn1) use 1-D Winograd F(2,3) along the
width: per 2 output cols, 4 transform points x 3 dy x 2 ih = 24 matmul
cycles/2px vs 36 direct (1.5x less PE work). Input transform V on DVE,
output transform A^T on DVE/ACT with fused bias. L5/L6 stay direct.
"""
import numpy as np

N_CORES = 8

# Quarter starts chosen so every upsample/subsample alignment offset is
# uniform across cores (SPMD requirement).
S2 = [0, 48, 96, 144]
S3 = [0, 24, 48, 72]
S4 = [0, 12, 24, 36]
S5 = [0, 6, 12, 18]
S6 = [0, 3, 6, 9]
H = [200, 100, 50, 25, 13]
W = [200, 100, 50, 25, 13]
OWN = [56, 28, 14, 7, 4]
VALID = [
    [48, 48, 48, 56],
    [24, 24, 24, 28],
    [12, 12, 12, 14],
    [6, 6, 6, 7],
    [3, 3, 3, 4],
]
M_ROWS = [60, 32, 18, 13]     # m2..m5 region rows (m_l from S_l-2, m5 S5-3)
CIN = [256, 512, 1024, 2048]
KH = [2, 4, 8, 16]
CW = [200, 100, 50, 26]       # c-slice widths (c5 x-padded to 26)
CONVW = [200, 100, 50, 26, 14]  # conv window x-counts (even)

_CACHED = {}


def _build(reps=1):
    import concourse.bacc as bacc
    import concourse.mybir as mybir
    from concourse.tile import TileContext

    F32, BF16 = mybir.dt.float32, mybir.dt.bfloat16
    AF = mybir.ActivationFunctionType
    ALU = mybir.AluOpType

    nc = bacc.Bacc("TRN2", target_bir_lowering=False, debug=False,
                   num_devices=N_CORES)

    c_d = [nc.dram_tensor(f"c{i}", [KH[i], 128, M_ROWS[i], CW[i]], BF16,
                          kind="ExternalInput") for i in range(4)]
    wlat_d = [nc.dram_tensor(f"wlat{i}", [KH[i], 128, 2, 128], BF16,
                             kind="ExternalInput") for i in range(4)]
    # direct 3x3 weights (L5 tr conv + L5/L6 rpn conv)
    wtr3_d = nc.dram_tensor("wtr3", [2, 128, 2, 3, 3, 128], BF16,
                            kind="ExternalInput")
    wrpn1_d = nc.dram_tensor("wrpn1", [2, 128, 2, 3, 3, 128], BF16,
                             kind="ExternalInput")
    # Winograd F(2,3) width-transformed weights (t axis of 4)
    wtrU_d = [nc.dram_tensor(f"wtrU{i}", [2, 128, 2, 3, 4, 128], BF16,
                             kind="ExternalInput") for i in range(3)]
    wrpnU_d = nc.dram_tensor("wrpnU", [2, 128, 2, 3, 4, 128], BF16,
                             kind="ExternalInput")
    wrpn2_d = nc.dram_tensor("wrpn2", [2, 128, 16], BF16,
                             kind="ExternalInput")
    btr_d = nc.dram_tensor("btr", [4, 128, 2], F32, kind="ExternalInput")
    brpn1_d = nc.dram_tensor("brpn1", [128, 2], F32, kind="ExternalInput")
    brpn2_d = nc.dram_tensor("brpn2", [16, 1], F32, kind="ExternalInput")
    zeros_d = nc.dram_tensor("zeros", [128, 512], BF16, kind="ExternalInput")
    masks_d = nc.dram_tensor("masks", [128, 244], BF16, kind="ExternalInput")
    MOFF = {}
    _off = 0
    for name, n in [("m2", 60), ("p2", 58), ("m3", 32), ("p3", 30),
                    ("m4", 18), ("p4", 16), ("m5", 13), ("p5", 11),
                    ("p6", 6)]:
        MOFF[name] = _off
        _off += n

    o_d = [nc.dram_tensor(f"o{i}", [OWN[i] * W[i] * 3 * 5], F32,
                          kind="ExternalOutput") for i in range(5)]

    with TileContext(nc, pool_alloc_mode="queue") as tc:
        with (
            tc.tile_pool(name="const", bufs=1) as cpool,
            tc.tile_pool(name="ps", bufs=4, space="PSUM") as pspool,
            tc.tile_pool(name="stg", bufs=1) as stgpool,
            tc.tile_pool(name="escr", bufs=2) as epool,
        ):
            tz = cpool.tile([128, 512], BF16, tag="zeros")
            tmask = cpool.tile([128, 244], BF16, tag="masks")
            twrU = cpool.tile([128, 2, 2, 3, 4, 128], BF16, tag="wrpnU")
            twr2 = cpool.tile([128, 2, 16], BF16, tag="wrpn2")
            tbtr = cpool.tile([128, 4, 2], F32, tag="btr")
            tbr1 = cpool.tile([128, 2], F32, tag="brpn1")
            tbr2 = cpool.tile([16, 1], F32, tag="brpn2")
            nc.gpsimd.dma_start(out=tz[:], in_=zeros_d[:])
            nc.gpsimd.dma_start(out=tmask[:], in_=masks_d[:])
            nc.gpsimd.dma_start(out=tbtr[:],
                                in_=btr_d.rearrange("l p t -> p l t"))
            nc.gpsimd.dma_start(out=tbr1[:], in_=brpn1_d[:])
            nc.gpsimd.dma_start(out=tbr2[:], in_=brpn2_d[:])
            for ih in range(2):
                nc.sync.dma_start(out=twr2[:, ih], in_=wrpn2_d[ih])
                for mo in range(2):
                    nc.sync.dma_start(out=twrU[:, ih, mo],
                                      in_=wrpnU_d[ih, :, mo])

            tm3 = cpool.tile([128, 2, 32, 102], BF16, tag="m3")
            tm4 = cpool.tile([128, 2, 18, 52], BF16, tag="m4")
            tm5 = cpool.tile([128, 2, 13, 28], BF16, tag="m5")

            def zero_cols(t, nrows, cols):
                for ih in range(2):
                    for c0 in cols:
                        nc.vector.tensor_copy(
                            t[:, ih, :, c0].squeeze(), tz[:, :nrows])

            def mask_rows(t, moff, rows, i0, i1):
                for ih in range(2):
                    for r in rows:
                        mk = tmask[:, moff + r:moff + r + 1].broadcast_to(
                            [128, i1 - i0])
                        nc.vector.tensor_tensor(
                            t[:, ih, r, i0:i1], t[:, ih, r, i0:i1], mk,
                            ALU.mult)

            def conv3x3(dst_tile, groups, src_tile, src_row_of, lhsT,
                        bias, relu, lvl, mask_edge=None):
                """Direct 3x3 conv (L5/L6): dst row j reads src rows
                j+src_row_of+dy. relu=True -> unpadded h tile, else padded
                p tile (interior cols 1..W). groups = [(j0, nrows)]."""
                wv, wl = CONVW[lvl], W[lvl]
                for (j0, nr) in groups:
                    for mo in range(2):
                        ps = pspool.tile([128, 2, 512], F32, tag="ps")
                        n = nr * wv
                        first = True
                        for ih in range(2):
                            for dy in range(3):
                                for dx in range(3):
                                    rhs = src_tile[
                                        :, ih,
                                        j0 + src_row_of + dy:
                                        j0 + src_row_of + dy + nr,
                                        dx:dx + wv]
                                    nc.tensor.matmul(
                                        ps[:, 0, :n], lhsT[:, ih, mo, dy, dx],
                                        rhs, start=first,
                                        stop=(ih == 1 and dy == 2 and dx == 2))
                                    first = False
                        psv = ps[:, 0, :n].rearrange("p (r x) -> p r x", x=wv)
                        if relu:
                            dst = dst_tile[:, mo, j0:j0 + nr, :wl]
                            nc.scalar.activation(dst, psv[:, :, :wl], AF.Relu,
                                                 bias=bias[:, mo:mo + 1])
                        else:
                            dst = dst_tile[:, mo, j0:j0 + nr, 1:1 + wl]
                            nc.scalar.activation(dst, psv[:, :, :wl],
                                                 AF.Identity,
                                                 bias=bias[:, mo:mo + 1])
                if mask_edge is not None:
                    nrt, moff = mask_edge
                    mask_rows(dst_tile, moff, [0, 1, nrt - 2, nrt - 1],
                              1, 1 + wl)

            def conv3x3_wino(dst_tile, groups, src_tile, src_row_of, Ut,
                             bias, relu, lvl, vpool, mask_edge=None):
                """1-D Winograd F(2,3) width conv. Same contract as conv3x3.

                V layout [128, ih, 4t, vr, Wg]; per group (j0, nr) and mo,
                P_t accumulates in PSUM bank t; A^T drain:
                  even(2g+1) = P0+P1+P2+b ; odd(2g+2) = P1-P2-P3+b.
                """
                wl = W[lvl]
                wg = wl // 2
                r0 = min(j0 for j0, _ in groups)
                r1 = max(j0 + nr for j0, nr in groups)
                vr = r1 - r0 + 2
                tv = vpool.tile([128, 2, 4, vr, wg], BF16, tag="v")
                sview = src_tile[
                    :, :, src_row_of + r0:src_row_of + r1 + 2,
                    :2 * wg + 2].rearrange("p i r (g s) -> p i r g s", s=2)
                for ih in range(2):
                    d0 = sview[:, ih, :, 0:wg, 0]
                    d1 = sview[:, ih, :, 0:wg, 1]
                    d2 = sview[:, ih, :, 1:wg + 1, 0]
                    d3 = sview[:, ih, :, 1:wg + 1, 1]
                    # split V work across DVE and the otherwise-idle GpSimd
                    nc.vector.tensor_tensor(tv[:, ih, 0], d0, d2,
                                            ALU.subtract)
                    nc.gpsimd.tensor_tensor(tv[:, ih, 1], d1, d2, ALU.add)
                    nc.vector.tensor_tensor(tv[:, ih, 2], d2, d1,
                                            ALU.subtract)
                    nc.gpsimd.tensor_tensor(tv[:, ih, 3], d1, d3,
                                            ALU.subtract)
                for (j0, nr) in groups:
                    n = nr * wg
                    for mo in range(2):
                        # two 2-bank tiles per group so slot releases are
                        # fine-grained: psA (t=1,3) retires early (C1/O
                        # readers), psB (t=0,2) late (final drain readers).
                        psA = pspool.tile([128, 2, 512], F32, tag="ps")
                        psB = pspool.tile([128, 2, 512], F32, tag="ps")
                        pbank = {1: psA[:, 0], 3: psA[:, 1],
                                 0: psB[:, 0], 2: psB[:, 1]}
                        for t in (1, 3, 0, 2):
                            for ih in range(2):
                                for dy in range(3):
                                    rhs = tv[:, ih, t,
                                             j0 - r0 + dy:j0 - r0 + dy + nr]
                                    nc.tensor.matmul(
                                        pbank[t][:, :n], Ut[:, ih, mo, dy, t],
                                        rhs, start=(ih == 0 and dy == 0),
                                        stop=(ih == 1 and dy == 2))
                        # drains: C1 = P1 + b; even = P0+C1+P2; odd = C1-P3-P2
                        tc1 = epool.tile([128, 512], F32, tag="c1")
                        te = epool.tile([128, 512], F32, tag="e")
                        to = epool.tile([128, 512], F32, tag="o")
                        nc.scalar.activation(tc1[:, :n], pbank[1][:, :n],
                                             AF.Identity,
                                             bias=bias[:, mo:mo + 1])
                        if relu:
                            dv = dst_tile[:, mo, j0:j0 + nr, :2 * wg
                                          ].rearrange(
                                "p r (g s) -> p r g s", s=2)
                            d_ev = dv[:, :, :, 0]
                            d_od = dv[:, :, :, 1]
                        else:
                            dv = dst_tile[:, mo, j0:j0 + nr, :2 * wg + 2
                                          ].rearrange(
                                "p r (g s) -> p r g s", s=2)
                            d_ev = dv[:, :, 0:wg, 1]
                            d_od = dv[:, :, 1:wg + 1, 0]
                        c1v = tc1[:, :n].rearrange("p (r g) -> p r g", g=wg)
                        ev = te[:, :n].rearrange("p (r g) -> p r g", g=wg)
                        ov = to[:, :n].rearrange("p (r g) -> p r g", g=wg)
                        p0 = pbank[0][:, :n].rearrange("p (r g) -> p r g",
                                                       g=wg)
                        p2 = pbank[2][:, :n].rearrange("p (r g) -> p r g",
                                                       g=wg)
                        p3 = pbank[3][:, :n].rearrange("p (r g) -> p r g",
                                                       g=wg)
                        nc.vector.tensor_tensor(ev, p0, c1v, ALU.add)
                        nc.vector.scalar_tensor_tensor(
                            ov, p3, -1.0, c1v, ALU.mult, ALU.add)
                        if relu:
                            tf = epool.tile([128, 512], F32, tag="f")
                            tf2 = epool.tile([128, 512], F32, tag="f")
                            fv = tf[:, :n].rearrange("p (r g) -> p r g", g=wg)
                            f2v = tf2[:, :n].rearrange("p (r g) -> p r g",
                                                       g=wg)
                            nc.vector.tensor_tensor(fv, ev, p2, ALU.add)
                            nc.vector.scalar_tensor_tensor(
                                f2v, p2, -1.0, ov, ALU.mult, ALU.add)
                            nc.scalar.activation(d_ev, fv, AF.Relu, bias=0.0)
                            nc.scalar.activation(d_od, f2v, AF.Relu, bias=0.0)
                        else:
                            nc.vector.tensor_tensor(d_ev, ev, p2, ALU.add)
                            nc.vector.scalar_tensor_tensor(
                                d_od, p2, -1.0, ov, ALU.mult, ALU.add)
                if mask_edge is not None:
                    nrt, moff = mask_edge
                    mask_rows(dst_tile, moff, [0, 1, nrt - 2, nrt - 1],
                              1, 1 + wl)

            def out_head(h_tile, npix, lvl, px0):
                # o = W2^T h, channel-major: twr2 stationary (16-col weight
                # loads), h streamed in 512-px chunks. Output layout in DRAM
                # is [ch=15, pix] per level; host transposes in _assemble.
                hflat = h_tile.rearrange("p t r x -> p t (r x)")
                tstage = stgpool.tile([16, 2800], F32, tag="ostage")
                for g in range((npix + 511) // 512):
                    p0 = g * 512
                    n = min(512, npix - p0)
                    pso = pspool.tile([128, 2, 512], F32, tag="ps")
                    for ih in range(2):
                        nc.tensor.matmul(
                            pso[:16, 0, :n], twr2[:, ih],
                            hflat[:, ih, p0:p0 + n],
                            start=(ih == 0), stop=(ih == 1))
                    nc.scalar.activation(tstage[:, p0:p0 + n],
                                         pso[:16, 0, :n],
                                         AF.Identity, bias=tbr2[:, 0:1])
                dst = o_d[lvl].rearrange("(c pix) -> c pix", c=15)[
                    :, px0:px0 + npix]
                nc.sync.dma_start(out=dst, in_=tstage[:15, :npix])

            # ================= L5 (+L6) =================
            for _rep in range(reps):
              with tc.tile_pool(name="l5", bufs=1) as pool:
                  tc5 = pool.tile([128, 16, 13, 26], BF16, tag="c5")
                  twl5 = pool.tile([128, 16, 2, 128], BF16, tag="wl5")
                  twt5 = pool.tile([128, 2, 2, 3, 3, 128], BF16, tag="wt5")
                  twr1 = pool.tile([128, 2, 2, 3, 3, 128], BF16, tag="wr1")
                  tp5 = pool.tile([128, 2, 11, 28], BF16, tag="p5")
                  tp6 = pool.tile([128, 2, 6, 16], BF16, tag="p6")
                  th5 = pool.tile([128, 2, 7, 25], BF16, tag="h5")
                  th6 = pool.tile([128, 2, 4, 13], BF16, tag="h6")
                  for kh in range(16):
                      nc.sync.dma_start(out=tc5[:, kh], in_=c_d[3][kh])
                      nc.sync.dma_start(out=twl5[:, kh], in_=wlat_d[3][kh])
                  for ih in range(2):
                      nc.sync.dma_start(out=twt5[:, ih], in_=wtr3_d[ih])
                      for mo in range(2):
                          nc.sync.dma_start(out=twr1[:, ih, mo],
                                            in_=wrpn1_d[ih, :, mo])
                  zero_cols(tm5, 13, [0, 26, 27])
                  zero_cols(tp5, 11, [0, 26, 27])
                  zero_cols(tp6, 6, [0, 14, 15])

                  ps = pspool.tile([128, 2, 512], F32, tag="ps")
                  for mo in range(2):
                      n = 13 * 26
                      for kh in range(16):
                          nc.tensor.matmul(
                              ps[:, mo, :n], twl5[:, kh, mo],
                              tc5[:, kh].rearrange("p r x -> p (r x)"),
                              start=(kh == 0), stop=(kh == 15))
                      psv = ps[:, mo, :n].rearrange("p (r x) -> p r x", x=26)
                      nc.scalar.activation(tm5[:, mo, :, 1:26], psv[:, :, :25],
                                           AF.Copy, bias=0.0)

                  conv3x3(tp5, [(0, 11)], tm5, 0, twt5, tbtr[:, 3], False, 3,
                          mask_edge=(11, MOFF["p5"]))
                  # p6 = p5[::2, ::2]: row j <- p5 row 2j, col x <- p5 col 1+2x
                  for ih in range(2):
                      for j in range(6):
                          src = tp5[:, ih, 2 * j, 1:27].rearrange(
                              "p (x s) -> p x s", s=2)[:, :, 0].squeeze()
                          nc.vector.tensor_copy(tp6[:, ih, j, 1:14], src)
                  conv3x3(th5, [(0, 7)], tp5, 1, twr1, tbr1, True, 3)
                  conv3x3(th6, [(0, 4)], tp6, 0, twr1, tbr1, True, 4)
                  out_head(th5, 7 * 25, 3, 0)
                  out_head(th6, 4 * 13, 4, 0)

              # ================= L4 =================
              with (tc.tile_pool(name="l4", bufs=1) as pool,
                  tc.tile_pool(name="v4", bufs=2) as vpool,
                  tc.tile_pool(name="c4chunk", bufs=3) as c4pool):
                  twl4 = pool.tile([128, 8, 2, 128], BF16, tag="wl4")
                  twt4 = pool.tile([128, 2, 2, 3, 4, 128], BF16, tag="wt4")
                  tp4 = pool.tile([128, 2, 16, 52], BF16, tag="p4")
                  th4 = pool.tile([128, 2, 14, 50], BF16, tag="h4")
                  for kh in range(8):
                      nc.sync.dma_start(out=twl4[:, kh], in_=wlat_d[2][kh])
                  for ih in range(2):
                      nc.sync.dma_start(out=twt4[:, ih], in_=wtrU_d[2][ih])
                  zero_cols(tm4, 18, [0, 51])
                  zero_cols(tp4, 16, [0, 51])

                  for (r0, nr) in [(0, 6), (6, 6), (12, 6)]:
                      tc4 = c4pool.tile([128, 8, 6, 50], BF16, tag="c4")
                      for kh in range(8):
                          nc.sync.dma_start(out=tc4[:, kh],
                                            in_=c_d[2][kh, :, r0:r0 + nr, :])
                      ps = pspool.tile([128, 2, 512], F32, tag="ps")
                      for mo in range(2):
                          n = nr * 50
                          for kh in range(8):
                              rhs = tc4[:, kh].rearrange("p r x -> p (r x)")
                              nc.tensor.matmul(ps[:, mo, :n], twl4[:, kh, mo],
                                               rhs,
                                               start=(kh == 0), stop=(kh == 7))
                          ps5 = ps[:, mo, :n].rearrange(
                              "p (hh r wh s) -> p r hh wh s", r=2, wh=25, s=2)
                          dest5 = tm4[:, mo, r0:r0 + nr, 1:51].rearrange(
                              "p (hh r) (wh s) -> p r hh wh s", r=2, s=2)
                          srow = 2 + r0 // 2
                          srcb = tm5[:, mo, srow:srow + nr // 2, 1:26
                                     ].unsqueeze(3).broadcast_to(
                              [128, nr // 2, 25, 2])
                          for par in range(2):
                              nc.vector.tensor_tensor(
                                  dest5[:, par], ps5[:, par], srcb, ALU.add)
                  conv3x3_wino(tp4, [(0, 16)], tm4, 0, twt4, tbtr[:, 2],
                               False, 2, vpool, mask_edge=(16, MOFF["p4"]))
                  conv3x3_wino(th4, [(0, 14)], tp4, 0, twrU, tbr1, True, 2,
                               vpool)
                  out_head(th4, 14 * 50, 2, 0)

              # ================= L3 =================
              with (tc.tile_pool(name="l3", bufs=1) as pool,
                  tc.tile_pool(name="v3", bufs=2) as vpool,
                  tc.tile_pool(name="p3pool", bufs=2) as p3pool,
                  tc.tile_pool(name="c3chunk", bufs=4) as c3pool):
                  twl3 = pool.tile([128, 4, 2, 128], BF16, tag="wl3")
                  twt3 = pool.tile([128, 2, 2, 3, 4, 128], BF16, tag="wt3")
                  th3 = pool.tile([128, 2, 14, 100], BF16, tag="h3")
                  for kh in range(4):
                      nc.sync.dma_start(out=twl3[:, kh], in_=wlat_d[1][kh])
                  for ih in range(2):
                      nc.sync.dma_start(out=twt3[:, ih], in_=wtrU_d[1][ih])
                  zero_cols(tm3, 32, [0, 101])

                  for ci in range(8):
                      r0 = ci * 4
                      if ci % 2 == 0:
                          tc3 = c3pool.tile([128, 4, 8, 100], BF16, tag="c3")
                          for kh in range(4):
                              nc.sync.dma_start(
                                  out=tc3[:, kh],
                                  in_=c_d[1][kh, :, r0:r0 + 8, :])
                      ps = pspool.tile([128, 2, 512], F32, tag="ps")
                      for mo in range(2):
                          for kh in range(4):
                              rhs = tc3[:, kh, (ci % 2) * 4:(ci % 2) * 4 + 4,
                                        :].rearrange("p r x -> p (r x)")
                              nc.tensor.matmul(ps[:, mo, :400],
                                               twl3[:, kh, mo],
                                               rhs, start=(kh == 0),
                                               stop=(kh == 3))
                          ps5 = ps[:, mo, :400].rearrange(
                              "p (hh r wh s) -> p r hh wh s", r=2, wh=50, s=2)
                          dest5 = tm3[:, mo, r0:r0 + 4, 1:101].rearrange(
                              "p (hh r) (wh s) -> p r hh wh s", r=2, s=2)
                          srow = 1 + r0 // 2
                          srcb = tm4[:, mo, srow:srow + 2, 1:51].unsqueeze(
                              3).broadcast_to([128, 2, 50, 2])
                          for par in range(2):
                              nc.vector.tensor_tensor(
                                  dest5[:, par], ps5[:, par], srcb, ALU.add)
                  # pipeline: both tr convs first, then rpn+head per s, so
                  # V-transforms for stage k compute while PE runs stage k-1
                  tp3s = []
                  for s in range(2):
                      tp3 = p3pool.tile([128, 2, 16, 102], BF16, tag="p3")
                      zero_cols(tp3, 16, [0, 101])
                      conv3x3_wino(tp3, [(0, 8), (8, 8)], tm3,
                                   14 * s, twt3, tbtr[:, 1], False, 1, vpool)
                      mask_rows(tp3, MOFF["p3"] + 14 * s, [0, 15], 1, 101)
                      tp3s.append(tp3)
                  for s in range(2):
                      conv3x3_wino(th3, [(0, 8), (8, 6)], tp3s[s], 0,
                                   twrU, tbr1, True, 1, vpool)
                      out_head(th3, 14 * 100, 1, s * 1400)

              # ================= L2 =================
              with (tc.tile_pool(name="l2", bufs=1) as pool,
                    tc.tile_pool(name="p2pool", bufs=2) as p2pool,
                    tc.tile_pool(name="v2", bufs=2) as vpool):
                  twl2 = pool.tile([128, 2, 2, 128], BF16, tag="wl2")
                  twt2 = pool.tile([128, 2, 2, 3, 4, 128], BF16, tag="wt2")
                  tm2 = pool.tile([128, 2, 18, 202], BF16, tag="m2")
                  th2 = pool.tile([128, 2, 14, 200], BF16, tag="h2")
                  for kh in range(2):
                      nc.sync.dma_start(out=twl2[:, kh], in_=wlat_d[0][kh])
                      nc.sync.dma_start(out=twt2[:, kh], in_=wtrU_d[0][kh])
                  zero_cols(tm2, 18, [0, 201])

                  def rpn_head2(s, tp2):
                      conv3x3_wino(th2, [(0, 5), (5, 5)],
                                   tp2, 0, twrU, tbr1, True, 0, vpool)
                      conv3x3_wino(th2, [(10, 4)],
                                   tp2, 0, twrU, tbr1, True, 0, vpool)
                      out_head(th2, 14 * 200, 0, s * 2800)

                  # 2-stage software pipeline: rpn/head lag the tr conv by
                  # two s-chunks so V-transforms hide under PE matmul work.
                  with tc.tile_pool(name="c2chunk", bufs=3) as c2pool:
                      pend = []
                      for s in range(4):
                          for ci in range(9):
                              r0 = ci * 2
                              tcc = c2pool.tile([128, 2, 2, 200], BF16,
                                                tag="c2")
                              for kh in range(2):
                                  nc.sync.dma_start(
                                      out=tcc[:, kh],
                                      in_=c_d[0][kh, :,
                                                 14 * s + r0:14 * s + r0 + 2,
                                                 :])
                              ps = pspool.tile([128, 2, 512], F32,
                                               tag="ps")
                              for mo in range(2):
                                  for kh in range(2):
                                      rhs = tcc[:, kh].rearrange(
                                          "p r x -> p (r x)")
                                      nc.tensor.matmul(
                                          ps[:, mo, :400], twl2[:, kh, mo],
                                          rhs,
                                          start=(kh == 0), stop=(kh == 1))
                                  ps4 = ps[:, mo, :400].rearrange(
                                      "p (r wh s) -> p r wh s",
                                      r=2, s=2)
                                  dest4 = tm2[:, mo, r0:r0 + 2, 1:201
                                              ].rearrange(
                                      "p r (wh s) -> p r wh s", s=2)
                                  srow = 1 + (14 * s + r0) // 2
                                  srcb = tm3[:, mo, srow, 1:101
                                             ].unsqueeze(1).unsqueeze(3)\
                                      .broadcast_to([128, 2, 100, 2])
                                  nc.vector.tensor_tensor(
                                      dest4, ps4, srcb, ALU.add)
                          if len(pend) >= 2:
                              rpn_head2(*pend.pop(0))
                          tp2 = p2pool.tile([128, 2, 16, 202], BF16,
                                            tag="p2")
                          zero_cols(tp2, 16, [0, 201])
                          conv3x3_wino(tp2, [(0, 4), (4, 4)],
                                       tm2, 0, twt2, tbtr[:, 0], False, 0,
                                       vpool)
                          conv3x3_wino(tp2, [(8, 4), (12, 4)],
                                       tm2, 0, twt2, tbtr[:, 0], False, 0,
                                       vpool)
                          mask_rows(tp2, MOFF["p2"] + 14 * s, [0, 15], 1, 201)
                          pend.append((s, tp2))
                      for item in pend:
                          rpn_head2(*item)

    nc.compile()
    return nc


def _prep_inputs(c2, c3, c4, c5, lat_w, tr_w, rpn_w1, rpn_b1, rpn_w2,
                 rpn_b2, lat_b, tr_b):
    import ml_dtypes
    bf16 = ml_dtypes.bfloat16
    cs = [c2, c3, c4, c5]
    base = {}

    def conv_w(w):
        # [O=256, I=256, 3, 3] -> [ih, 128k, mo, dy, dx, 128m]
        return np.ascontiguousarray(
            w.reshape(2, 128, 2, 128, 3, 3).transpose(2, 3, 0, 4, 5, 1)
        ).astype(bf16)

    G = np.array([[1, 0, 0], [0.5, 0.5, 0.5], [0.5, -0.5, 0.5], [0, 0, 1]],
                 np.float32)

    def conv_wU(w):
        # Winograd F(2,3) width: U[o,i,dy,t] = sum_dx G[t,dx] w[o,i,dy,dx]
        U = np.einsum("oiyx,tx->oiyt", w.astype(np.float32), G)
        return np.ascontiguousarray(
            U.reshape(2, 128, 2, 128, 3, 4).transpose(2, 3, 0, 4, 5, 1)
        ).astype(bf16)

    for i in range(4):
        base[f"wlat{i}"] = np.ascontiguousarray(
            lat_w[i].reshape(2, 128, KH[i], 128).transpose(2, 3, 0, 1)
        ).astype(bf16)
    for i in range(3):
        base[f"wtrU{i}"] = conv_wU(tr_w[i])
    base["wtr3"] = conv_w(tr_w[3])
    base["wrpn1"] = conv_w(rpn_w1)
    base["wrpnU"] = conv_wU(rpn_w1)
    w2 = np.zeros((2, 128, 16), np.float32)
    w2[:, :, :15] = rpn_w2.reshape(15, 2, 128).transpose(1, 2, 0)
    base["wrpn2"] = w2.astype(bf16)
    base["btr"] = np.ascontiguousarray(
        np.stack([b.reshape(2, 128).T for b in tr_b]).transpose(0, 1, 2))
    base["brpn1"] = np.ascontiguousarray(rpn_b1.reshape(2, 128).T)
    b2 = np.zeros((16, 1), np.float32)
    b2[:15, 0] = rpn_b2
    base["brpn2"] = b2
    base["zeros"] = np.zeros((128, 512), bf16)
    for b in lat_b:
        if np.abs(b).max() != 0:
            raise NotImplementedError("nonzero lateral bias not supported")

    in_maps = []
    starts = [S2, S3, S4, S5]
    m_off = [2, 2, 2, 3]
    for b in range(2):
        for q in range(4):
            m = dict(base)
            for i in range(4):
                r0 = starts[i][q] - m_off[i]
                rows = M_ROWS[i]
                sl = np.zeros((CIN[i], rows, CW[i]), np.float32)
                lo, hi = max(0, r0), min(H[i], r0 + rows)
                if hi > lo:
                    sl[:, lo - r0:hi - r0, :W[i]] = cs[i][b, :, lo:hi, :]
                m[f"c{i}"] = np.ascontiguousarray(
                    sl.reshape(KH[i], 128, rows, CW[i])).astype(bf16)
            mk = np.zeros(244, np.float32)
            spans = [(S2[q] - 2, 200, 60), (S2[q] - 1, 200, 58),
                     (S3[q] - 2, 100, 32), (S3[q] - 1, 100, 30),
                     (S4[q] - 2, 50, 18), (S4[q] - 1, 50, 16),
                     (S5[q] - 3, 25, 13), (S5[q] - 2, 25, 11),
                     (S6[q] - 1, 13, 6)]
            off = 0
            for a0, hh, ln in spans:
                for j in range(ln):
                    mk[off + j] = 1.0 if 0 <= a0 + j < hh else 0.0
                off += ln
            m["masks"] = np.broadcast_to(mk, (128, 244)).astype(bf16)
            in_maps.append(m)
    return in_maps


def _assemble(results):
    out = np.zeros((2, 159882, 5), np.float32)
    lvl_off = [0, 120000, 150000, 157500, 159375]
    starts = [S2, S3, S4, S5, S6]
    for b in range(2):
        for q in range(4):
            r = results[b * 4 + q]
            for lv in range(5):
                v, w = VALID[lv][q], W[lv]
                # device layout is [ch=15, pix]; transpose to pixel-major
                o = np.ascontiguousarray(
                    r[f"o{lv}"].reshape(15, OWN[lv] * w).T
                ).reshape(OWN[lv] * w * 3, 5)
                a0 = lvl_off[lv] + starts[lv][q] * w * 3
                out[b, a0:a0 + v * w * 3] = o[:v * w * 3]
    return out


def kernel(**inputs):
    import os
    from concourse.bass_utils import run_bass_kernel_spmd
    if "nc" not in _CACHED:
        _CACHED["nc"] = _build(int(os.environ.get("K_REPS", "1")))
    nc = _CACHED["nc"]
    in_maps = _prep_inputs(
        np.asarray(inputs["c2"], np.float32),
        np.asarray(inputs["c3"], np.float32),
        np.asarray(inputs["c4"], np.float32),
        np.asarray(inputs["c5"], np.float32),
        [np.asarray(inputs[f"lat_w{i}"], np.float32) for i in range(4)],
        [np.asarray(inputs[f"tr_w{i}"], np.float32) for i in range(4)],
        np.asarray(inputs["rpn_w1"], np.float32),
        np.asarray(inputs["rpn_b1"], np.float32),
        np.asarray(inputs["rpn_w2"], np.float32),
        np.asarray(inputs["rpn_b2"], np.float32),
        [np.asarray(inputs[f"lat_b{i}"], np.float32) for i in range(4)],
        [np.asarray(inputs[f"tr_b{i}"], np.float32) for i in range(4)],
    )
    res = run_bass_kernel_spmd(nc, in_maps, list(range(N_CORES)))
    return _assemble(res.results)


# revision 16
# speedup vs baseline: 1.1539x; 1.0024x over previous
"""FPN + RPN detector kernel for Trainium2, 8-core SPMD.

Sharding: core = (image b in {0,1}) x (row-quarter q in {0..3}). Host slices
inputs with halos (zero-padded), every core runs the same program on its
slice, host crops/reassembles. No cross-core communication.

All convs are bf16 matmuls (full PE rate at any N, fast weight load),
channels on partitions in two halves of 128, PSUM f32 accumulation over
taps/halves; biases and outputs stay f32.

The 3x3 convs at L2/L3/L4 (tr + rpn1) use 1-D Winograd F(2,3) along the
width: per 2 output cols, 4 transform points x 3 dy x 2 ih = 24 matmul
cycles/2px vs 36 direct (1.5x less PE work). Input transform V on DVE,
output transform A^T on DVE/ACT with fused bias. L5/L6 stay direct.
"""
import numpy as np

N_CORES = 8

# Quarter starts chosen so every upsample/subsample alignment offset is
# uniform across cores (SPMD requirement).
S2 = [0, 48, 96, 144]
S3 = [0, 24, 48, 72]
S4 = [0, 12, 24, 36]
S5 = [0, 6, 12, 18]
S6 = [0, 3, 6, 9]
H = [200, 100, 50, 25, 13]
W = [200, 100, 50, 25, 13]
OWN = [56, 28, 14, 7, 4]
VALID = [
    [48, 48, 48, 56],
    [24, 24, 24, 28],
    [12, 12, 12, 14],
    [6, 6, 6, 7],
    [3, 3, 3, 4],
]
M_ROWS = [60, 32, 18, 13]     # m2..m5 region rows (m_l from S_l-2, m5 S5-3)
CIN = [256, 512, 1024, 2048]
KH = [2, 4, 8, 16]
CW = [200, 100, 50, 26]       # c-slice widths (c5 x-padded to 26)
CONVW = [200, 100, 50, 26, 14]  # conv window x-counts (even)

_CACHED = {}


def _build(reps=1):
    import concourse.bacc as bacc
    import concourse.mybir as mybir
    from concourse.tile import TileContext

    F32, BF16 = mybir.dt.float32, mybir.dt.bfloat16
    AF = mybir.ActivationFunctionType
    ALU = mybir.AluOpType

    nc = bacc.Bacc("TRN2", target_bir_lowering=False, debug=False,
                   num_devices=N_CORES)

    c_d = [nc.dram_tensor(f"c{i}", [KH[i], 128, M_ROWS[i], CW[i]], BF16,
                          kind="ExternalInput") for i in range(4)]
    wlat_d = [nc.dram_tensor(f"wlat{i}", [KH[i], 128, 2, 128], BF16,
                             kind="ExternalInput") for i in range(4)]
    # direct 3x3 weights (L5 tr conv + L5/L6 rpn conv)
    wtr3_d = nc.dram_tensor("wtr3", [2, 128, 2, 3, 3, 128], BF16,
                            kind="ExternalInput")
    wrpn1_d = nc.dram_tensor("wrpn1", [2, 128, 2, 3, 3, 128], BF16,
                             kind="ExternalInput")
    # Winograd F(2,3) width-transformed weights (t axis of 4)
    wtrU_d = [nc.dram_tensor(f"wtrU{i}", [2, 128, 2, 3, 4, 128], BF16,
                             kind="ExternalInput") for i in range(3)]
    wrpnU_d = nc.dram_tensor("wrpnU", [2, 128, 2, 3, 4, 128], BF16,
                             kind="ExternalInput")
    wrpn2_d = nc.dram_tensor("wrpn2", [2, 128, 16], BF16,
                             kind="ExternalInput")
    btr_d = nc.dram_tensor("btr", [4, 128, 2], F32, kind="ExternalInput")
    brpn1_d = nc.dram_tensor("brpn1", [128, 2], F32, kind="ExternalInput")
    brpn2_d = nc.dram_tensor("brpn2", [16, 1], F32, kind="ExternalInput")
    zeros_d = nc.dram_tensor("zeros", [128, 512], BF16, kind="ExternalInput")
    masks_d = nc.dram_tensor("masks", [128, 244], BF16, kind="ExternalInput")
    MOFF = {}
    _off = 0
    for name, n in [("m2", 60), ("p2", 58), ("m3", 32), ("p3", 30),
                    ("m4", 18), ("p4", 16), ("m5", 13), ("p5", 11),
                    ("p6", 6)]:
        MOFF[name] = _off
        _off += n

    o_d = [nc.dram_tensor(f"o{i}", [OWN[i] * W[i] * 3 * 5], F32,
                          kind="ExternalOutput") for i in range(5)]

    with TileContext(nc, pool_alloc_mode="queue") as tc:
        with (
            tc.tile_pool(name="const", bufs=1) as cpool,
            tc.tile_pool(name="ps", bufs=4, space="PSUM") as pspool,
            tc.tile_pool(name="stg", bufs=1) as stgpool,
            tc.tile_pool(name="escr", bufs=2) as epool,
        ):
            tz = cpool.tile([128, 512], BF16, tag="zeros")
            tmask = cpool.tile([128, 244], BF16, tag="masks")
            twrU = cpool.tile([128, 2, 2, 3, 4, 128], BF16, tag="wrpnU")
            twr2 = cpool.tile([128, 2, 16], BF16, tag="wrpn2")
            tbtr = cpool.tile([128, 4, 2], F32, tag="btr")
            tbr1 = cpool.tile([128, 2], F32, tag="brpn1")
            tbr2 = cpool.tile([16, 1], F32, tag="brpn2")
            nc.gpsimd.dma_start(out=tz[:], in_=zeros_d[:])
            nc.gpsimd.dma_start(out=tmask[:], in_=masks_d[:])
            nc.gpsimd.dma_start(out=tbtr[:],
                                in_=btr_d.rearrange("l p t -> p l t"))
            nc.gpsimd.dma_start(out=tbr1[:], in_=brpn1_d[:])
            nc.gpsimd.dma_start(out=tbr2[:], in_=brpn2_d[:])
            for ih in range(2):
                nc.sync.dma_start(out=twr2[:, ih], in_=wrpn2_d[ih])
                for mo in range(2):
                    nc.sync.dma_start(out=twrU[:, ih, mo],
                                      in_=wrpnU_d[ih, :, mo])

            tm3 = cpool.tile([128, 2, 32, 102], BF16, tag="m3")
            tm4 = cpool.tile([128, 2, 18, 52], BF16, tag="m4")
            tm5 = cpool.tile([128, 2, 13, 28], BF16, tag="m5")

            def zero_cols(t, nrows, cols):
                for ih in range(2):
                    for c0 in cols:
                        nc.vector.tensor_copy(
                            t[:, ih, :, c0].squeeze(), tz[:, :nrows])

            def mask_rows(t, moff, rows, i0, i1):
                for ih in range(2):
                    for r in rows:
                        mk = tmask[:, moff + r:moff + r + 1].broadcast_to(
                            [128, i1 - i0])
                        nc.vector.tensor_tensor(
                            t[:, ih, r, i0:i1], t[:, ih, r, i0:i1], mk,
                            ALU.mult)

            def conv3x3(dst_tile, groups, src_tile, src_row_of, lhsT,
                        bias, relu, lvl, mask_edge=None):
                """Direct 3x3 conv (L5/L6): dst row j reads src rows
                j+src_row_of+dy. relu=True -> unpadded h tile, else padded
                p tile (interior cols 1..W). groups = [(j0, nrows)]."""
                wv, wl = CONVW[lvl], W[lvl]
                for (j0, nr) in groups:
                    for mo in range(2):
                        ps = pspool.tile([128, 2, 512], F32, tag="ps")
                        n = nr * wv
                        first = True
                        for ih in range(2):
                            for dy in range(3):
                                for dx in range(3):
                                    rhs = src_tile[
                                        :, ih,
                                        j0 + src_row_of + dy:
                                        j0 + src_row_of + dy + nr,
                                        dx:dx + wv]
                                    nc.tensor.matmul(
                                        ps[:, 0, :n], lhsT[:, ih, mo, dy, dx],
                                        rhs, start=first,
                                        stop=(ih == 1 and dy == 2 and dx == 2))
                                    first = False
                        psv = ps[:, 0, :n].rearrange("p (r x) -> p r x", x=wv)
                        if relu:
                            dst = dst_tile[:, mo, j0:j0 + nr, :wl]
                            nc.scalar.activation(dst, psv[:, :, :wl], AF.Relu,
                                                 bias=bias[:, mo:mo + 1])
                        else:
                            dst = dst_tile[:, mo, j0:j0 + nr, 1:1 + wl]
                            nc.scalar.activation(dst, psv[:, :, :wl],
                                                 AF.Identity,
                                                 bias=bias[:, mo:mo + 1])
                if mask_edge is not None:
                    nrt, moff = mask_edge
                    mask_rows(dst_tile, moff, [0, 1, nrt - 2, nrt - 1],
                              1, 1 + wl)

            def conv3x3_wino(dst_tile, groups, src_tile, src_row_of, Ut,
                             bias, relu, lvl, vpool, mask_edge=None):
                """1-D Winograd F(2,3) width conv. Same contract as conv3x3.

                V layout [128, ih, 4t, vr, Wg]; per group (j0, nr) and mo,
                P_t accumulates in PSUM bank t; A^T drain:
                  even(2g+1) = P0+P1+P2+b ; odd(2g+2) = P1-P2-P3+b.
                """
                wl = W[lvl]
                wg = wl // 2
                r0 = min(j0 for j0, _ in groups)
                r1 = max(j0 + nr for j0, nr in groups)
                vr = r1 - r0 + 2
                tv = vpool.tile([128, 2, 4, vr, wg], BF16, tag="v")
                sview = src_tile[
                    :, :, src_row_of + r0:src_row_of + r1 + 2,
                    :2 * wg + 2].rearrange("p i r (g s) -> p i r g s", s=2)
                for ih in range(2):
                    d0 = sview[:, ih, :, 0:wg, 0]
                    d1 = sview[:, ih, :, 0:wg, 1]
                    d2 = sview[:, ih, :, 1:wg + 1, 0]
                    d3 = sview[:, ih, :, 1:wg + 1, 1]
                    # split V work across DVE and the otherwise-idle GpSimd
                    nc.vector.tensor_tensor(tv[:, ih, 0], d0, d2,
                                            ALU.subtract)
                    nc.vector.tensor_tensor(tv[:, ih, 1], d1, d2, ALU.add)
                    nc.vector.tensor_tensor(tv[:, ih, 2], d2, d1,
                                            ALU.subtract)
                    nc.gpsimd.tensor_tensor(tv[:, ih, 3], d1, d3,
                                            ALU.subtract)
                for (j0, nr) in groups:
                    n = nr * wg
                    for mo in range(2):
                        # two 2-bank tiles per group so slot releases are
                        # fine-grained: psA (t=1,3) retires early (C1/O
                        # readers), psB (t=0,2) late (final drain readers).
                        psA = pspool.tile([128, 2, 512], F32, tag="ps")
                        psB = pspool.tile([128, 2, 512], F32, tag="ps")
                        pbank = {1: psA[:, 0], 3: psA[:, 1],
                                 0: psB[:, 0], 2: psB[:, 1]}
                        for t in (1, 3, 0, 2):
                            for ih in range(2):
                                for dy in range(3):
                                    rhs = tv[:, ih, t,
                                             j0 - r0 + dy:j0 - r0 + dy + nr]
                                    nc.tensor.matmul(
                                        pbank[t][:, :n], Ut[:, ih, mo, dy, t],
                                        rhs, start=(ih == 0 and dy == 0),
                                        stop=(ih == 1 and dy == 2))
                        # drains: C1 = P1 + b; even = P0+C1+P2; odd = C1-P3-P2
                        tc1 = epool.tile([128, 512], F32, tag="c1")
                        te = epool.tile([128, 512], F32, tag="e")
                        to = epool.tile([128, 512], F32, tag="o")
                        nc.scalar.activation(tc1[:, :n], pbank[1][:, :n],
                                             AF.Identity,
                                             bias=bias[:, mo:mo + 1])
                        if relu:
                            dv = dst_tile[:, mo, j0:j0 + nr, :2 * wg
                                          ].rearrange(
                                "p r (g s) -> p r g s", s=2)
                            d_ev = dv[:, :, :, 0]
                            d_od = dv[:, :, :, 1]
                        else:
                            dv = dst_tile[:, mo, j0:j0 + nr, :2 * wg + 2
                                          ].rearrange(
                                "p r (g s) -> p r g s", s=2)
                            d_ev = dv[:, :, 0:wg, 1]
                            d_od = dv[:, :, 1:wg + 1, 0]
                        c1v = tc1[:, :n].rearrange("p (r g) -> p r g", g=wg)
                        ev = te[:, :n].rearrange("p (r g) -> p r g", g=wg)
                        ov = to[:, :n].rearrange("p (r g) -> p r g", g=wg)
                        p0 = pbank[0][:, :n].rearrange("p (r g) -> p r g",
                                                       g=wg)
                        p2 = pbank[2][:, :n].rearrange("p (r g) -> p r g",
                                                       g=wg)
                        p3 = pbank[3][:, :n].rearrange("p (r g) -> p r g",
                                                       g=wg)
                        nc.vector.tensor_tensor(ev, p0, c1v, ALU.add)
                        nc.vector.scalar_tensor_tensor(
                            ov, p3, -1.0, c1v, ALU.mult, ALU.add)
                        if relu:
                            tf = epool.tile([128, 512], F32, tag="f")
                            tf2 = epool.tile([128, 512], F32, tag="f")
                            fv = tf[:, :n].rearrange("p (r g) -> p r g", g=wg)
                            f2v = tf2[:, :n].rearrange("p (r g) -> p r g",
                                                       g=wg)
                            nc.vector.tensor_tensor(fv, ev, p2, ALU.add)
                            nc.vector.scalar_tensor_tensor(
                                f2v, p2, -1.0, ov, ALU.mult, ALU.add)
                            nc.scalar.activation(d_ev, fv, AF.Relu, bias=0.0)
                            nc.scalar.activation(d_od, f2v, AF.Relu, bias=0.0)
                        else:
                            nc.vector.tensor_tensor(d_ev, ev, p2, ALU.add)
                            nc.vector.scalar_tensor_tensor(
                                d_od, p2, -1.0, ov, ALU.mult, ALU.add)
                if mask_edge is not None:
                    nrt, moff = mask_edge
                    mask_rows(dst_tile, moff, [0, 1, nrt - 2, nrt - 1],
                              1, 1 + wl)

            def out_head(h_tile, npix, lvl, px0):
                # o = W2^T h, channel-major: twr2 stationary (16-col weight
                # loads), h streamed in 512-px chunks. Output layout in DRAM
                # is [ch=15, pix] per level; host transposes in _assemble.
                hflat = h_tile.rearrange("p t r x -> p t (r x)")
                tstage = stgpool.tile([16, 2800], F32, tag="ostage")
                for g in range((npix + 511) // 512):
                    p0 = g * 512
                    n = min(512, npix - p0)
                    pso = pspool.tile([128, 2, 512], F32, tag="ps")
                    for ih in range(2):
                        nc.tensor.matmul(
                            pso[:16, 0, :n], twr2[:, ih],
                            hflat[:, ih, p0:p0 + n],
                            start=(ih == 0), stop=(ih == 1))
                    nc.scalar.activation(tstage[:, p0:p0 + n],
                                         pso[:16, 0, :n],
                                         AF.Identity, bias=tbr2[:, 0:1])
                dst = o_d[lvl].rearrange("(c pix) -> c pix", c=15)[
                    :, px0:px0 + npix]
                nc.sync.dma_start(out=dst, in_=tstage[:15, :npix])

            # ================= L5 (+L6) =================
            for _rep in range(reps):
              with tc.tile_pool(name="l5", bufs=1) as pool:
                  tc5 = pool.tile([128, 16, 13, 26], BF16, tag="c5")
                  twl5 = pool.tile([128, 16, 2, 128], BF16, tag="wl5")
                  twt5 = pool.tile([128, 2, 2, 3, 3, 128], BF16, tag="wt5")
                  twr1 = pool.tile([128, 2, 2, 3, 3, 128], BF16, tag="wr1")
                  tp5 = pool.tile([128, 2, 11, 28], BF16, tag="p5")
                  tp6 = pool.tile([128, 2, 6, 16], BF16, tag="p6")
                  th5 = pool.tile([128, 2, 7, 25], BF16, tag="h5")
                  th6 = pool.tile([128, 2, 4, 13], BF16, tag="h6")
                  nc.sync.dma_start(out=tc5[:],
                                    in_=c_d[3].rearrange("k p r x -> p k r x"))
                  nc.sync.dma_start(
                      out=twl5[:],
                      in_=wlat_d[3].rearrange("k p m o -> p k m o"))
                  nc.sync.dma_start(
                      out=twt5[:],
                      in_=wtr3_d.rearrange("i p m y x o -> p i m y x o"))
                  nc.sync.dma_start(
                      out=twr1[:],
                      in_=wrpn1_d.rearrange("i p m y x o -> p i m y x o"))
                  zero_cols(tm5, 13, [0, 26, 27])
                  zero_cols(tp5, 11, [0, 26, 27])
                  zero_cols(tp6, 6, [0, 14, 15])

                  ps = pspool.tile([128, 2, 512], F32, tag="ps")
                  for mo in range(2):
                      n = 13 * 26
                      for kh in range(16):
                          nc.tensor.matmul(
                              ps[:, mo, :n], twl5[:, kh, mo],
                              tc5[:, kh].rearrange("p r x -> p (r x)"),
                              start=(kh == 0), stop=(kh == 15))
                      psv = ps[:, mo, :n].rearrange("p (r x) -> p r x", x=26)
                      nc.scalar.activation(tm5[:, mo, :, 1:26], psv[:, :, :25],
                                           AF.Copy, bias=0.0)

                  conv3x3(tp5, [(0, 11)], tm5, 0, twt5, tbtr[:, 3], False, 3,
                          mask_edge=(11, MOFF["p5"]))
                  # p6 = p5[::2, ::2]: row j <- p5 row 2j, col x <- p5 col 1+2x
                  for ih in range(2):
                      for j in range(6):
                          src = tp5[:, ih, 2 * j, 1:27].rearrange(
                              "p (x s) -> p x s", s=2)[:, :, 0].squeeze()
                          nc.vector.tensor_copy(tp6[:, ih, j, 1:14], src)
                  conv3x3(th5, [(0, 7)], tp5, 1, twr1, tbr1, True, 3)
                  conv3x3(th6, [(0, 4)], tp6, 0, twr1, tbr1, True, 4)
                  out_head(th5, 7 * 25, 3, 0)
                  out_head(th6, 4 * 13, 4, 0)

              # ================= L4 =================
              with (tc.tile_pool(name="l4", bufs=1) as pool,
                  tc.tile_pool(name="v4", bufs=2) as vpool,
                  tc.tile_pool(name="c4chunk", bufs=3) as c4pool):
                  twl4 = pool.tile([128, 8, 2, 128], BF16, tag="wl4")
                  twt4 = pool.tile([128, 2, 2, 3, 4, 128], BF16, tag="wt4")
                  tp4 = pool.tile([128, 2, 16, 52], BF16, tag="p4")
                  th4 = pool.tile([128, 2, 14, 50], BF16, tag="h4")
                  nc.sync.dma_start(
                      out=twl4[:],
                      in_=wlat_d[2].rearrange("k p m o -> p k m o"))
                  nc.sync.dma_start(
                      out=twt4[:],
                      in_=wtrU_d[2].rearrange("i p m y t o -> p i m y t o"))
                  zero_cols(tm4, 18, [0, 51])
                  zero_cols(tp4, 16, [0, 51])

                  for (r0, nr) in [(0, 6), (6, 6), (12, 6)]:
                      tc4 = c4pool.tile([128, 8, 6, 50], BF16, tag="c4")
                      nc.sync.dma_start(
                          out=tc4[:],
                          in_=c_d[2][:, :, r0:r0 + nr, :].rearrange(
                              "k p r x -> p k r x"))
                      ps = pspool.tile([128, 2, 512], F32, tag="ps")
                      for mo in range(2):
                          n = nr * 50
                          for kh in range(8):
                              rhs = tc4[:, kh].rearrange("p r x -> p (r x)")
                              nc.tensor.matmul(ps[:, mo, :n], twl4[:, kh, mo],
                                               rhs,
                                               start=(kh == 0), stop=(kh == 7))
                          ps5 = ps[:, mo, :n].rearrange(
                              "p (hh r wh s) -> p r hh wh s", r=2, wh=25, s=2)
                          dest5 = tm4[:, mo, r0:r0 + nr, 1:51].rearrange(
                              "p (hh r) (wh s) -> p r hh wh s", r=2, s=2)
                          srow = 2 + r0 // 2
                          srcb = tm5[:, mo, srow:srow + nr // 2, 1:26
                                     ].unsqueeze(3).broadcast_to(
                              [128, nr // 2, 25, 2])
                          for par in range(2):
                              nc.vector.tensor_tensor(
                                  dest5[:, par], ps5[:, par], srcb, ALU.add)
                  conv3x3_wino(tp4, [(0, 16)], tm4, 0, twt4, tbtr[:, 2],
                               False, 2, vpool, mask_edge=(16, MOFF["p4"]))
                  conv3x3_wino(th4, [(0, 14)], tp4, 0, twrU, tbr1, True, 2,
                               vpool)
                  out_head(th4, 14 * 50, 2, 0)

              # ================= L3 =================
              with (tc.tile_pool(name="l3", bufs=1) as pool,
                  tc.tile_pool(name="v3", bufs=2) as vpool,
                  tc.tile_pool(name="p3pool", bufs=2) as p3pool,
                  tc.tile_pool(name="c3chunk", bufs=4) as c3pool):
                  twl3 = pool.tile([128, 4, 2, 128], BF16, tag="wl3")
                  twt3 = pool.tile([128, 2, 2, 3, 4, 128], BF16, tag="wt3")
                  th3 = pool.tile([128, 2, 14, 100], BF16, tag="h3")
                  nc.sync.dma_start(
                      out=twl3[:],
                      in_=wlat_d[1].rearrange("k p m o -> p k m o"))
                  nc.sync.dma_start(
                      out=twt3[:],
                      in_=wtrU_d[1].rearrange("i p m y t o -> p i m y t o"))
                  zero_cols(tm3, 32, [0, 101])

                  for ci in range(8):
                      r0 = ci * 4
                      if ci % 2 == 0:
                          tc3 = c3pool.tile([128, 4, 8, 100], BF16, tag="c3")
                          nc.sync.dma_start(
                              out=tc3[:],
                              in_=c_d[1][:, :, r0:r0 + 8, :].rearrange(
                                  "k p r x -> p k r x"))
                      ps = pspool.tile([128, 2, 512], F32, tag="ps")
                      for mo in range(2):
                          for kh in range(4):
                              rhs = tc3[:, kh, (ci % 2) * 4:(ci % 2) * 4 + 4,
                                        :].rearrange("p r x -> p (r x)")
                              nc.tensor.matmul(ps[:, mo, :400],
                                               twl3[:, kh, mo],
                                               rhs, start=(kh == 0),
                                               stop=(kh == 3))
                          ps5 = ps[:, mo, :400].rearrange(
                              "p (hh r wh s) -> p r hh wh s", r=2, wh=50, s=2)
                          dest5 = tm3[:, mo, r0:r0 + 4, 1:101].rearrange(
                              "p (hh r) (wh s) -> p r hh wh s", r=2, s=2)
                          srow = 1 + r0 // 2
                          srcb = tm4[:, mo, srow:srow + 2, 1:51].unsqueeze(
                              3).broadcast_to([128, 2, 50, 2])
                          for par in range(2):
                              nc.vector.tensor_tensor(
                                  dest5[:, par], ps5[:, par], srcb, ALU.add)
                  # pipeline: both tr convs first, then rpn+head per s, so
                  # V-transforms for stage k compute while PE runs stage k-1
                  tp3s = []
                  for s in range(2):
                      tp3 = p3pool.tile([128, 2, 16, 102], BF16, tag="p3")
                      zero_cols(tp3, 16, [0, 101])
                      conv3x3_wino(tp3, [(0, 8), (8, 8)], tm3,
                                   14 * s, twt3, tbtr[:, 1], False, 1, vpool)
                      mask_rows(tp3, MOFF["p3"] + 14 * s, [0, 15], 1, 101)
                      tp3s.append(tp3)
                  for s in range(2):
                      conv3x3_wino(th3, [(0, 8), (8, 6)], tp3s[s], 0,
                                   twrU, tbr1, True, 1, vpool)
                      out_head(th3, 14 * 100, 1, s * 1400)

              # ================= L2 =================
              with (tc.tile_pool(name="l2", bufs=1) as pool,
                    tc.tile_pool(name="p2pool", bufs=2) as p2pool,
                    tc.tile_pool(name="v2", bufs=2) as vpool):
                  twl2 = pool.tile([128, 2, 2, 128], BF16, tag="wl2")
                  twt2 = pool.tile([128, 2, 2, 3, 4, 128], BF16, tag="wt2")
                  tm2 = pool.tile([128, 2, 18, 202], BF16, tag="m2")
                  th2 = pool.tile([128, 2, 14, 200], BF16, tag="h2")
                  nc.sync.dma_start(
                      out=twl2[:],
                      in_=wlat_d[0].rearrange("k p m o -> p k m o"))
                  nc.sync.dma_start(
                      out=twt2[:],
                      in_=wtrU_d[0].rearrange("i p m y t o -> p i m y t o"))
                  zero_cols(tm2, 18, [0, 201])

                  def rpn_head2(s, tp2):
                      conv3x3_wino(th2, [(0, 5), (5, 5)],
                                   tp2, 0, twrU, tbr1, True, 0, vpool)
                      conv3x3_wino(th2, [(10, 4)],
                                   tp2, 0, twrU, tbr1, True, 0, vpool)
                      out_head(th2, 14 * 200, 0, s * 2800)

                  # 2-stage software pipeline: rpn/head lag the tr conv by
                  # two s-chunks so V-transforms hide under PE matmul work.
                  with tc.tile_pool(name="c2chunk", bufs=3) as c2pool:
                      pend = []
                      for s in range(4):
                          for ci in range(9):
                              r0 = ci * 2
                              tcc = c2pool.tile([128, 2, 2, 200], BF16,
                                                tag="c2")
                              nc.sync.dma_start(
                                  out=tcc[:],
                                  in_=c_d[0][:, :,
                                             14 * s + r0:14 * s + r0 + 2,
                                             :].rearrange(
                                      "k p r x -> p k r x"))
                              ps = pspool.tile([128, 2, 512], F32,
                                               tag="ps")
                              for mo in range(2):
                                  for kh in range(2):
                                      rhs = tcc[:, kh].rearrange(
                                          "p r x -> p (r x)")
                                      nc.tensor.matmul(
                                          ps[:, mo, :400], twl2[:, kh, mo],
                                          rhs,
                                          start=(kh == 0), stop=(kh == 1))
                                  ps4 = ps[:, mo, :400].rearrange(
                                      "p (r wh s) -> p r wh s",
                                      r=2, s=2)
                                  dest4 = tm2[:, mo, r0:r0 + 2, 1:201
                                              ].rearrange(
                                      "p r (wh s) -> p r wh s", s=2)
                                  srow = 1 + (14 * s + r0) // 2
                                  srcb = tm3[:, mo, srow, 1:101
                                             ].unsqueeze(1).unsqueeze(3)\
                                      .broadcast_to([128, 2, 100, 2])
                                  nc.vector.tensor_tensor(
                                      dest4, ps4, srcb, ALU.add)
                          if len(pend) >= 2:
                              rpn_head2(*pend.pop(0))
                          tp2 = p2pool.tile([128, 2, 16, 202], BF16,
                                            tag="p2")
                          zero_cols(tp2, 16, [0, 201])
                          conv3x3_wino(tp2, [(0, 4), (4, 4)],
                                       tm2, 0, twt2, tbtr[:, 0], False, 0,
                                       vpool)
                          conv3x3_wino(tp2, [(8, 4), (12, 4)],
                                       tm2, 0, twt2, tbtr[:, 0], False, 0,
                                       vpool)
                          mask_rows(tp2, MOFF["p2"] + 14 * s, [0, 15], 1, 201)
                          pend.append((s, tp2))
                      for item in pend:
                          rpn_head2(*item)

    nc.compile()
    return nc


def _prep_inputs(c2, c3, c4, c5, lat_w, tr_w, rpn_w1, rpn_b1, rpn_w2,
                 rpn_b2, lat_b, tr_b):
    import ml_dtypes
    bf16 = ml_dtypes.bfloat16
    cs = [c2, c3, c4, c5]
    base = {}

    def conv_w(w):
        # [O=256, I=256, 3, 3] -> [ih, 128k, mo, dy, dx, 128m]
        return np.ascontiguousarray(
            w.reshape(2, 128, 2, 128, 3, 3).transpose(2, 3, 0, 4, 5, 1)
        ).astype(bf16)

    G = np.array([[1, 0, 0], [0.5, 0.5, 0.5], [0.5, -0.5, 0.5], [0, 0, 1]],
                 np.float32)

    def conv_wU(w):
        # Winograd F(2,3) width: U[o,i,dy,t] = sum_dx G[t,dx] w[o,i,dy,dx]
        U = np.einsum("oiyx,tx->oiyt", w.astype(np.float32), G)
        return np.ascontiguousarray(
            U.reshape(2, 128, 2, 128, 3, 4).transpose(2, 3, 0, 4, 5, 1)
        ).astype(bf16)

    for i in range(4):
        base[f"wlat{i}"] = np.ascontiguousarray(
            lat_w[i].reshape(2, 128, KH[i], 128).transpose(2, 3, 0, 1)
        ).astype(bf16)
    for i in range(3):
        base[f"wtrU{i}"] = conv_wU(tr_w[i])
    base["wtr3"] = conv_w(tr_w[3])
    base["wrpn1"] = conv_w(rpn_w1)
    base["wrpnU"] = conv_wU(rpn_w1)
    w2 = np.zeros((2, 128, 16), np.float32)
    w2[:, :, :15] = rpn_w2.reshape(15, 2, 128).transpose(1, 2, 0)
    base["wrpn2"] = w2.astype(bf16)
    base["btr"] = np.ascontiguousarray(
        np.stack([b.reshape(2, 128).T for b in tr_b]).transpose(0, 1, 2))
    base["brpn1"] = np.ascontiguousarray(rpn_b1.reshape(2, 128).T)
    b2 = np.zeros((16, 1), np.float32)
    b2[:15, 0] = rpn_b2
    base["brpn2"] = b2
    base["zeros"] = np.zeros((128, 512), bf16)
    for b in lat_b:
        if np.abs(b).max() != 0:
            raise NotImplementedError("nonzero lateral bias not supported")

    in_maps = []
    starts = [S2, S3, S4, S5]
    m_off = [2, 2, 2, 3]
    for b in range(2):
        for q in range(4):
            m = dict(base)
            for i in range(4):
                r0 = starts[i][q] - m_off[i]
                rows = M_ROWS[i]
                sl = np.zeros((CIN[i], rows, CW[i]), np.float32)
                lo, hi = max(0, r0), min(H[i], r0 + rows)
                if hi > lo:
                    sl[:, lo - r0:hi - r0, :W[i]] = cs[i][b, :, lo:hi, :]
                m[f"c{i}"] = np.ascontiguousarray(
                    sl.reshape(KH[i], 128, rows, CW[i])).astype(bf16)
            mk = np.zeros(244, np.float32)
            spans = [(S2[q] - 2, 200, 60), (S2[q] - 1, 200, 58),
                     (S3[q] - 2, 100, 32), (S3[q] - 1, 100, 30),
                     (S4[q] - 2, 50, 18), (S4[q] - 1, 50, 16),
                     (S5[q] - 3, 25, 13), (S5[q] - 2, 25, 11),
                     (S6[q] - 1, 13, 6)]
            off = 0
            for a0, hh, ln in spans:
                for j in range(ln):
                    mk[off + j] = 1.0 if 0 <= a0 + j < hh else 0.0
                off += ln
            m["masks"] = np.broadcast_to(mk, (128, 244)).astype(bf16)
            in_maps.append(m)
    return in_maps


def _assemble(results):
    out = np.zeros((2, 159882, 5), np.float32)
    lvl_off = [0, 120000, 150000, 157500, 159375]
    starts = [S2, S3, S4, S5, S6]
    for b in range(2):
        for q in range(4):
            r = results[b * 4 + q]
            for lv in range(5):
                v, w = VALID[lv][q], W[lv]
                # device layout is [ch=15, pix]; transpose to pixel-major
                o = np.ascontiguousarray(
                    r[f"o{lv}"].reshape(15, OWN[lv] * w).T
                ).reshape(OWN[lv] * w * 3, 5)
                a0 = lvl_off[lv] + starts[lv][q] * w * 3
                out[b, a0:a0 + v * w * 3] = o[:v * w * 3]
    return out


def kernel(**inputs):
    import os
    from concourse.bass_utils import run_bass_kernel_spmd
    if "nc" not in _CACHED:
        _CACHED["nc"] = _build(int(os.environ.get("K_REPS", "1")))
    nc = _CACHED["nc"]
    in_maps = _prep_inputs(
        np.asarray(inputs["c2"], np.float32),
        np.asarray(inputs["c3"], np.float32),
        np.asarray(inputs["c4"], np.float32),
        np.asarray(inputs["c5"], np.float32),
        [np.asarray(inputs[f"lat_w{i}"], np.float32) for i in range(4)],
        [np.asarray(inputs[f"tr_w{i}"], np.float32) for i in range(4)],
        np.asarray(inputs["rpn_w1"], np.float32),
        np.asarray(inputs["rpn_b1"], np.float32),
        np.asarray(inputs["rpn_w2"], np.float32),
        np.asarray(inputs["rpn_b2"], np.float32),
        [np.asarray(inputs[f"lat_b{i}"], np.float32) for i in range(4)],
        [np.asarray(inputs[f"tr_b{i}"], np.float32) for i in range(4)],
    )
    res = run_bass_kernel_spmd(nc, in_maps, list(range(N_CORES)))
    return _assemble(res.results)


# revision 17
# speedup vs baseline: 1.1800x; 1.0226x over previous
"""FPN + RPN detector kernel for Trainium2, 8-core SPMD.

Sharding: core = (image b in {0,1}) x (row-quarter q in {0..3}). Host slices
inputs with halos (zero-padded), every core runs the same program on its
slice, host crops/reassembles. No cross-core communication.

All convs are bf16 matmuls (full PE rate at any N, fast weight load),
channels on partitions in two halves of 128, PSUM f32 accumulation over
taps/halves; biases and outputs stay f32.

The 3x3 convs at L2/L3/L4 (tr + rpn1) use 1-D Winograd F(2,3) along the
width: per 2 output cols, 4 transform points x 3 dy x 2 ih = 24 matmul
cycles/2px vs 36 direct (1.5x less PE work). Input transform V on DVE,
output transform A^T on DVE/ACT with fused bias. L5/L6 stay direct.
"""
import numpy as np

N_CORES = 8

# Quarter starts chosen so every upsample/subsample alignment offset is
# uniform across cores (SPMD requirement).
S2 = [0, 48, 96, 144]
S3 = [0, 24, 48, 72]
S4 = [0, 12, 24, 36]
S5 = [0, 6, 12, 18]
S6 = [0, 3, 6, 9]
H = [200, 100, 50, 25, 13]
W = [200, 100, 50, 25, 13]
OWN = [56, 28, 14, 7, 4]
VALID = [
    [48, 48, 48, 56],
    [24, 24, 24, 28],
    [12, 12, 12, 14],
    [6, 6, 6, 7],
    [3, 3, 3, 4],
]
M_ROWS = [60, 32, 18, 13]     # m2..m5 region rows (m_l from S_l-2, m5 S5-3)
CIN = [256, 512, 1024, 2048]
KH = [2, 4, 8, 16]
CW = [200, 100, 50, 26]       # c-slice widths (c5 x-padded to 26)
CONVW = [200, 100, 50, 26, 14]  # conv window x-counts (even)

_CACHED = {}


def _build(reps=1):
    import concourse.bacc as bacc
    import concourse.mybir as mybir
    from concourse.tile import TileContext

    F32, BF16 = mybir.dt.float32, mybir.dt.bfloat16
    AF = mybir.ActivationFunctionType
    ALU = mybir.AluOpType

    nc = bacc.Bacc("TRN2", target_bir_lowering=False, debug=False,
                   num_devices=N_CORES)

    c_d = [nc.dram_tensor(f"c{i}", [KH[i], 128, M_ROWS[i], CW[i]], BF16,
                          kind="ExternalInput") for i in range(4)]
    wlat_d = [nc.dram_tensor(f"wlat{i}", [KH[i], 128, 2, 128], BF16,
                             kind="ExternalInput") for i in range(4)]
    # direct 3x3 weights (L5 tr conv + L5/L6 rpn conv)
    wtr3_d = nc.dram_tensor("wtr3", [2, 128, 2, 3, 3, 128], BF16,
                            kind="ExternalInput")
    wrpn1_d = nc.dram_tensor("wrpn1", [2, 128, 2, 3, 3, 128], BF16,
                             kind="ExternalInput")
    # Winograd F(2,3) width-transformed weights (t axis of 4)
    wtrU_d = [nc.dram_tensor(f"wtrU{i}", [2, 128, 2, 3, 4, 128], BF16,
                             kind="ExternalInput") for i in range(3)]
    wrpnU_d = nc.dram_tensor("wrpnU", [2, 128, 2, 3, 4, 128], BF16,
                             kind="ExternalInput")
    wrpn2_d = nc.dram_tensor("wrpn2", [2, 128, 16], BF16,
                             kind="ExternalInput")
    btr_d = nc.dram_tensor("btr", [4, 128, 2], F32, kind="ExternalInput")
    brpn1_d = nc.dram_tensor("brpn1", [128, 2], F32, kind="ExternalInput")
    brpn2_d = nc.dram_tensor("brpn2", [16, 1], F32, kind="ExternalInput")
    zeros_d = nc.dram_tensor("zeros", [128, 512], BF16, kind="ExternalInput")
    masks_d = nc.dram_tensor("masks", [128, 244], BF16, kind="ExternalInput")
    MOFF = {}
    _off = 0
    for name, n in [("m2", 60), ("p2", 58), ("m3", 32), ("p3", 30),
                    ("m4", 18), ("p4", 16), ("m5", 13), ("p5", 11),
                    ("p6", 6)]:
        MOFF[name] = _off
        _off += n

    o_d = [nc.dram_tensor(f"o{i}", [OWN[i] * W[i] * 3 * 5], F32,
                          kind="ExternalOutput") for i in range(5)]

    with TileContext(nc, pool_alloc_mode="queue") as tc:
        with (
            tc.tile_pool(name="const", bufs=1) as cpool,
            tc.tile_pool(name="ps", bufs=4, space="PSUM") as pspool,
            tc.tile_pool(name="stg", bufs=1) as stgpool,
            tc.tile_pool(name="escr", bufs=2) as epool,
            tc.tile_pool(name="c5p", bufs=2) as c5pool,
        ):
            tz = cpool.tile([128, 512], BF16, tag="zeros")
            tmask = cpool.tile([128, 244], BF16, tag="masks")
            twrU = cpool.tile([128, 2, 2, 3, 4, 128], BF16, tag="wrpnU")
            twr2 = cpool.tile([128, 2, 16], BF16, tag="wrpn2")
            tbtr = cpool.tile([128, 4, 2], F32, tag="btr")
            tbr1 = cpool.tile([128, 2], F32, tag="brpn1")
            tbr2 = cpool.tile([16, 1], F32, tag="brpn2")
            nc.gpsimd.dma_start(out=tz[:], in_=zeros_d[:])
            nc.gpsimd.dma_start(out=tmask[:], in_=masks_d[:])
            nc.gpsimd.dma_start(out=tbtr[:],
                                in_=btr_d.rearrange("l p t -> p l t"))
            nc.gpsimd.dma_start(out=tbr1[:], in_=brpn1_d[:])
            nc.gpsimd.dma_start(out=tbr2[:], in_=brpn2_d[:])
            for ih in range(2):
                nc.sync.dma_start(out=twr2[:, ih], in_=wrpn2_d[ih])
                for mo in range(2):
                    nc.sync.dma_start(out=twrU[:, ih, mo],
                                      in_=wrpnU_d[ih, :, mo])

            tm3 = cpool.tile([128, 2, 32, 102], BF16, tag="m3")
            tm4 = cpool.tile([128, 2, 18, 52], BF16, tag="m4")
            tm5 = cpool.tile([128, 2, 13, 28], BF16, tag="m5")

            def zero_cols(t, nrows, cols):
                for ih in range(2):
                    for c0 in cols:
                        nc.vector.tensor_copy(
                            t[:, ih, :, c0].squeeze(), tz[:, :nrows])

            def mask_rows(t, moff, rows, i0, i1):
                for ih in range(2):
                    for r in rows:
                        mk = tmask[:, moff + r:moff + r + 1].broadcast_to(
                            [128, i1 - i0])
                        nc.vector.tensor_tensor(
                            t[:, ih, r, i0:i1], t[:, ih, r, i0:i1], mk,
                            ALU.mult)

            def conv3x3(dst_tile, groups, src_tile, src_row_of, lhsT,
                        bias, relu, lvl, mask_edge=None):
                """Direct 3x3 conv (L5/L6): dst row j reads src rows
                j+src_row_of+dy. relu=True -> unpadded h tile, else padded
                p tile (interior cols 1..W). groups = [(j0, nrows)]."""
                wv, wl = CONVW[lvl], W[lvl]
                for (j0, nr) in groups:
                    for mo in range(2):
                        ps = pspool.tile([128, 2, 512], F32, tag="ps")
                        n = nr * wv
                        first = True
                        for ih in range(2):
                            for dy in range(3):
                                for dx in range(3):
                                    rhs = src_tile[
                                        :, ih,
                                        j0 + src_row_of + dy:
                                        j0 + src_row_of + dy + nr,
                                        dx:dx + wv]
                                    nc.tensor.matmul(
                                        ps[:, 0, :n], lhsT[:, ih, mo, dy, dx],
                                        rhs, start=first,
                                        stop=(ih == 1 and dy == 2 and dx == 2))
                                    first = False
                        psv = ps[:, 0, :n].rearrange("p (r x) -> p r x", x=wv)
                        if relu:
                            dst = dst_tile[:, mo, j0:j0 + nr, :wl]
                            nc.scalar.activation(dst, psv[:, :, :wl], AF.Relu,
                                                 bias=bias[:, mo:mo + 1])
                        else:
                            dst = dst_tile[:, mo, j0:j0 + nr, 1:1 + wl]
                            nc.scalar.activation(dst, psv[:, :, :wl],
                                                 AF.Identity,
                                                 bias=bias[:, mo:mo + 1])
                if mask_edge is not None:
                    nrt, moff = mask_edge
                    mask_rows(dst_tile, moff, [0, 1, nrt - 2, nrt - 1],
                              1, 1 + wl)

            def conv3x3_wino(dst_tile, groups, src_tile, src_row_of, Ut,
                             bias, relu, lvl, vpool, mask_edge=None):
                """1-D Winograd F(2,3) width conv. Same contract as conv3x3.

                V layout [128, ih, 4t, vr, Wg]; per group (j0, nr) and mo,
                P_t accumulates in PSUM bank t; A^T drain:
                  even(2g+1) = P0+P1+P2+b ; odd(2g+2) = P1-P2-P3+b.
                """
                wl = W[lvl]
                wg = wl // 2
                r0 = min(j0 for j0, _ in groups)
                r1 = max(j0 + nr for j0, nr in groups)
                vr = r1 - r0 + 2
                tv = vpool.tile([128, 2, 4, vr, wg], BF16, tag="v")
                sview = src_tile[
                    :, :, src_row_of + r0:src_row_of + r1 + 2,
                    :2 * wg + 2].rearrange("p i r (g s) -> p i r g s", s=2)
                for ih in range(2):
                    d0 = sview[:, ih, :, 0:wg, 0]
                    d1 = sview[:, ih, :, 0:wg, 1]
                    d2 = sview[:, ih, :, 1:wg + 1, 0]
                    d3 = sview[:, ih, :, 1:wg + 1, 1]
                    # split V work across DVE and the otherwise-idle GpSimd
                    nc.vector.tensor_tensor(tv[:, ih, 0], d0, d2,
                                            ALU.subtract)
                    nc.vector.tensor_tensor(tv[:, ih, 1], d1, d2, ALU.add)
                    nc.vector.tensor_tensor(tv[:, ih, 2], d2, d1,
                                            ALU.subtract)
                    nc.gpsimd.tensor_tensor(tv[:, ih, 3], d1, d3,
                                            ALU.subtract)
                for (j0, nr) in groups:
                    n = nr * wg
                    for mo in range(2):
                        # two 2-bank tiles per group so slot releases are
                        # fine-grained: psA (t=1,3) retires early (C1/O
                        # readers), psB (t=0,2) late (final drain readers).
                        psA = pspool.tile([128, 2, 512], F32, tag="ps")
                        psB = pspool.tile([128, 2, 512], F32, tag="ps")
                        pbank = {1: psA[:, 0], 3: psA[:, 1],
                                 0: psB[:, 0], 2: psB[:, 1]}
                        for t in (1, 3, 0, 2):
                            for ih in range(2):
                                for dy in range(3):
                                    rhs = tv[:, ih, t,
                                             j0 - r0 + dy:j0 - r0 + dy + nr]
                                    nc.tensor.matmul(
                                        pbank[t][:, :n], Ut[:, ih, mo, dy, t],
                                        rhs, start=(ih == 0 and dy == 0),
                                        stop=(ih == 1 and dy == 2))
                        # drains: C1 = P1 + b; even = P0+C1+P2; odd = C1-P3-P2
                        tc1 = epool.tile([128, 512], F32, tag="c1")
                        te = epool.tile([128, 512], F32, tag="e")
                        to = epool.tile([128, 512], F32, tag="o")
                        nc.scalar.activation(tc1[:, :n], pbank[1][:, :n],
                                             AF.Identity,
                                             bias=bias[:, mo:mo + 1])
                        if relu:
                            dv = dst_tile[:, mo, j0:j0 + nr, :2 * wg
                                          ].rearrange(
                                "p r (g s) -> p r g s", s=2)
                            d_ev = dv[:, :, :, 0]
                            d_od = dv[:, :, :, 1]
                        else:
                            dv = dst_tile[:, mo, j0:j0 + nr, :2 * wg + 2
                                          ].rearrange(
                                "p r (g s) -> p r g s", s=2)
                            d_ev = dv[:, :, 0:wg, 1]
                            d_od = dv[:, :, 1:wg + 1, 0]
                        c1v = tc1[:, :n].rearrange("p (r g) -> p r g", g=wg)
                        ev = te[:, :n].rearrange("p (r g) -> p r g", g=wg)
                        ov = to[:, :n].rearrange("p (r g) -> p r g", g=wg)
                        p0 = pbank[0][:, :n].rearrange("p (r g) -> p r g",
                                                       g=wg)
                        p2 = pbank[2][:, :n].rearrange("p (r g) -> p r g",
                                                       g=wg)
                        p3 = pbank[3][:, :n].rearrange("p (r g) -> p r g",
                                                       g=wg)
                        nc.vector.tensor_tensor(ev, p0, c1v, ALU.add)
                        nc.vector.scalar_tensor_tensor(
                            ov, p3, -1.0, c1v, ALU.mult, ALU.add)
                        if relu:
                            tf = epool.tile([128, 512], F32, tag="f")
                            tf2 = epool.tile([128, 512], F32, tag="f")
                            fv = tf[:, :n].rearrange("p (r g) -> p r g", g=wg)
                            f2v = tf2[:, :n].rearrange("p (r g) -> p r g",
                                                       g=wg)
                            nc.vector.tensor_tensor(fv, ev, p2, ALU.add)
                            nc.vector.scalar_tensor_tensor(
                                f2v, p2, -1.0, ov, ALU.mult, ALU.add)
                            nc.scalar.activation(d_ev, fv, AF.Relu, bias=0.0)
                            nc.scalar.activation(d_od, f2v, AF.Relu, bias=0.0)
                        else:
                            nc.vector.tensor_tensor(d_ev, ev, p2, ALU.add)
                            nc.vector.scalar_tensor_tensor(
                                d_od, p2, -1.0, ov, ALU.mult, ALU.add)
                if mask_edge is not None:
                    nrt, moff = mask_edge
                    mask_rows(dst_tile, moff, [0, 1, nrt - 2, nrt - 1],
                              1, 1 + wl)

            def out_head(h_tile, npix, lvl, px0):
                # o = W2^T h, channel-major: twr2 stationary (16-col weight
                # loads), h streamed in 512-px chunks. Output layout in DRAM
                # is [ch=15, pix] per level; host transposes in _assemble.
                hflat = h_tile.rearrange("p t r x -> p t (r x)")
                tstage = stgpool.tile([16, 2800], F32, tag="ostage")
                for g in range((npix + 511) // 512):
                    p0 = g * 512
                    n = min(512, npix - p0)
                    pso = pspool.tile([128, 2, 512], F32, tag="ps")
                    for ih in range(2):
                        nc.tensor.matmul(
                            pso[:16, 0, :n], twr2[:, ih],
                            hflat[:, ih, p0:p0 + n],
                            start=(ih == 0), stop=(ih == 1))
                    nc.scalar.activation(tstage[:, p0:p0 + n],
                                         pso[:16, 0, :n],
                                         AF.Identity, bias=tbr2[:, 0:1])
                dst = o_d[lvl].rearrange("(c pix) -> c pix", c=15)[
                    :, px0:px0 + npix]
                nc.sync.dma_start(out=dst, in_=tstage[:15, :npix])

            # ================= L5 (+L6) =================
            for _rep in range(reps):
              with tc.tile_pool(name="l5", bufs=1) as pool:
                  tc5 = c5pool.tile([128, 16, 13, 26], BF16, tag="c5")
                  twl5 = pool.tile([128, 16, 2, 128], BF16, tag="wl5")
                  twt5 = pool.tile([128, 2, 2, 3, 3, 128], BF16, tag="wt5")
                  twr1 = pool.tile([128, 2, 2, 3, 3, 128], BF16, tag="wr1")
                  tp5 = pool.tile([128, 2, 11, 28], BF16, tag="p5")
                  tp6 = pool.tile([128, 2, 6, 16], BF16, tag="p6")
                  th5 = pool.tile([128, 2, 7, 25], BF16, tag="h5")
                  th6 = pool.tile([128, 2, 4, 13], BF16, tag="h6")
                  nc.sync.dma_start(out=tc5[:],
                                    in_=c_d[3].rearrange("k p r x -> p k r x"))
                  nc.sync.dma_start(
                      out=twl5[:],
                      in_=wlat_d[3].rearrange("k p m o -> p k m o"))
                  nc.sync.dma_start(
                      out=twt5[:],
                      in_=wtr3_d.rearrange("i p m y x o -> p i m y x o"))
                  nc.sync.dma_start(
                      out=twr1[:],
                      in_=wrpn1_d.rearrange("i p m y x o -> p i m y x o"))
                  zero_cols(tm5, 13, [0, 26, 27])
                  zero_cols(tp5, 11, [0, 26, 27])
                  zero_cols(tp6, 6, [0, 14, 15])

                  ps = pspool.tile([128, 2, 512], F32, tag="ps")
                  for mo in range(2):
                      n = 13 * 26
                      for kh in range(16):
                          nc.tensor.matmul(
                              ps[:, mo, :n], twl5[:, kh, mo],
                              tc5[:, kh].rearrange("p r x -> p (r x)"),
                              start=(kh == 0), stop=(kh == 15))
                      psv = ps[:, mo, :n].rearrange("p (r x) -> p r x", x=26)
                      nc.scalar.activation(tm5[:, mo, :, 1:26], psv[:, :, :25],
                                           AF.Copy, bias=0.0)

                  conv3x3(tp5, [(0, 11)], tm5, 0, twt5, tbtr[:, 3], False, 3,
                          mask_edge=(11, MOFF["p5"]))
                  # p6 = p5[::2, ::2]: row j <- p5 row 2j, col x <- p5 col 1+2x
                  for ih in range(2):
                      for j in range(6):
                          src = tp5[:, ih, 2 * j, 1:27].rearrange(
                              "p (x s) -> p x s", s=2)[:, :, 0].squeeze()
                          nc.vector.tensor_copy(tp6[:, ih, j, 1:14], src)
                  conv3x3(th5, [(0, 7)], tp5, 1, twr1, tbr1, True, 3)
                  conv3x3(th6, [(0, 4)], tp6, 0, twr1, tbr1, True, 4)
                  out_head(th5, 7 * 25, 3, 0)
                  out_head(th6, 4 * 13, 4, 0)

              # ================= L4 =================
              with (tc.tile_pool(name="l4", bufs=1) as pool,
                  tc.tile_pool(name="v4", bufs=2) as vpool,
                  tc.tile_pool(name="c4chunk", bufs=3) as c4pool):
                  twl4 = pool.tile([128, 8, 2, 128], BF16, tag="wl4")
                  twt4 = pool.tile([128, 2, 2, 3, 4, 128], BF16, tag="wt4")
                  tp4 = pool.tile([128, 2, 16, 52], BF16, tag="p4")
                  th4 = pool.tile([128, 2, 14, 50], BF16, tag="h4")
                  nc.sync.dma_start(
                      out=twl4[:],
                      in_=wlat_d[2].rearrange("k p m o -> p k m o"))
                  nc.sync.dma_start(
                      out=twt4[:],
                      in_=wtrU_d[2].rearrange("i p m y t o -> p i m y t o"))
                  zero_cols(tm4, 18, [0, 51])
                  zero_cols(tp4, 16, [0, 51])

                  for (r0, nr) in [(0, 6), (6, 6), (12, 6)]:
                      tc4 = c4pool.tile([128, 8, 6, 50], BF16, tag="c4")
                      nc.sync.dma_start(
                          out=tc4[:],
                          in_=c_d[2][:, :, r0:r0 + nr, :].rearrange(
                              "k p r x -> p k r x"))
                      ps = pspool.tile([128, 2, 512], F32, tag="ps")
                      for mo in range(2):
                          n = nr * 50
                          for kh in range(8):
                              rhs = tc4[:, kh].rearrange("p r x -> p (r x)")
                              nc.tensor.matmul(ps[:, mo, :n], twl4[:, kh, mo],
                                               rhs,
                                               start=(kh == 0), stop=(kh == 7))
                          ps5 = ps[:, mo, :n].rearrange(
                              "p (hh r wh s) -> p r hh wh s", r=2, wh=25, s=2)
                          dest5 = tm4[:, mo, r0:r0 + nr, 1:51].rearrange(
                              "p (hh r) (wh s) -> p r hh wh s", r=2, s=2)
                          srow = 2 + r0 // 2
                          srcb = tm5[:, mo, srow:srow + nr // 2, 1:26
                                     ].unsqueeze(3).broadcast_to(
                              [128, nr // 2, 25, 2])
                          for par in range(2):
                              nc.vector.tensor_tensor(
                                  dest5[:, par], ps5[:, par], srcb, ALU.add)
                  conv3x3_wino(tp4, [(0, 16)], tm4, 0, twt4, tbtr[:, 2],
                               False, 2, vpool, mask_edge=(16, MOFF["p4"]))
                  conv3x3_wino(th4, [(0, 14)], tp4, 0, twrU, tbr1, True, 2,
                               vpool)
                  out_head(th4, 14 * 50, 2, 0)

              # ================= L3 =================
              with (tc.tile_pool(name="l3", bufs=1) as pool,
                  tc.tile_pool(name="v3", bufs=2) as vpool,
                  tc.tile_pool(name="p3pool", bufs=2) as p3pool,
                  tc.tile_pool(name="c3chunk", bufs=4) as c3pool):
                  twl3 = pool.tile([128, 4, 2, 128], BF16, tag="wl3")
                  twt3 = pool.tile([128, 2, 2, 3, 4, 128], BF16, tag="wt3")
                  th3 = pool.tile([128, 2, 14, 100], BF16, tag="h3")
                  nc.sync.dma_start(
                      out=twl3[:],
                      in_=wlat_d[1].rearrange("k p m o -> p k m o"))
                  nc.sync.dma_start(
                      out=twt3[:],
                      in_=wtrU_d[1].rearrange("i p m y t o -> p i m y t o"))
                  zero_cols(tm3, 32, [0, 101])

                  for ci in range(8):
                      r0 = ci * 4
                      if ci % 2 == 0:
                          tc3 = c3pool.tile([128, 4, 8, 100], BF16, tag="c3")
                          nc.sync.dma_start(
                              out=tc3[:],
                              in_=c_d[1][:, :, r0:r0 + 8, :].rearrange(
                                  "k p r x -> p k r x"))
                      ps = pspool.tile([128, 2, 512], F32, tag="ps")
                      for mo in range(2):
                          for kh in range(4):
                              rhs = tc3[:, kh, (ci % 2) * 4:(ci % 2) * 4 + 4,
                                        :].rearrange("p r x -> p (r x)")
                              nc.tensor.matmul(ps[:, mo, :400],
                                               twl3[:, kh, mo],
                                               rhs, start=(kh == 0),
                                               stop=(kh == 3))
                          ps5 = ps[:, mo, :400].rearrange(
                              "p (hh r wh s) -> p r hh wh s", r=2, wh=50, s=2)
                          dest5 = tm3[:, mo, r0:r0 + 4, 1:101].rearrange(
                              "p (hh r) (wh s) -> p r hh wh s", r=2, s=2)
                          srow = 1 + r0 // 2
                          srcb = tm4[:, mo, srow:srow + 2, 1:51].unsqueeze(
                              3).broadcast_to([128, 2, 50, 2])
                          for par in range(2):
                              nc.vector.tensor_tensor(
                                  dest5[:, par], ps5[:, par], srcb, ALU.add)
                  # pipeline: both tr convs first, then rpn+head per s, so
                  # V-transforms for stage k compute while PE runs stage k-1
                  tp3s = []
                  for s in range(2):
                      tp3 = p3pool.tile([128, 2, 16, 102], BF16, tag="p3")
                      zero_cols(tp3, 16, [0, 101])
                      conv3x3_wino(tp3, [(0, 8), (8, 8)], tm3,
                                   14 * s, twt3, tbtr[:, 1], False, 1, vpool)
                      mask_rows(tp3, MOFF["p3"] + 14 * s, [0, 15], 1, 101)
                      tp3s.append(tp3)
                  for s in range(2):
                      conv3x3_wino(th3, [(0, 8), (8, 6)], tp3s[s], 0,
                                   twrU, tbr1, True, 1, vpool)
                      out_head(th3, 14 * 100, 1, s * 1400)

              # ================= L2 =================
              with (tc.tile_pool(name="l2", bufs=1) as pool,
                    tc.tile_pool(name="p2pool", bufs=2) as p2pool,
                    tc.tile_pool(name="v2", bufs=2) as vpool):
                  twl2 = pool.tile([128, 2, 2, 128], BF16, tag="wl2")
                  twt2 = pool.tile([128, 2, 2, 3, 4, 128], BF16, tag="wt2")
                  tm2 = pool.tile([128, 2, 18, 202], BF16, tag="m2")
                  th2 = pool.tile([128, 2, 14, 200], BF16, tag="h2")
                  nc.sync.dma_start(
                      out=twl2[:],
                      in_=wlat_d[0].rearrange("k p m o -> p k m o"))
                  nc.sync.dma_start(
                      out=twt2[:],
                      in_=wtrU_d[0].rearrange("i p m y t o -> p i m y t o"))
                  zero_cols(tm2, 18, [0, 201])

                  def rpn_head2(s, tp2):
                      conv3x3_wino(th2, [(0, 5), (5, 5)],
                                   tp2, 0, twrU, tbr1, True, 0, vpool)
                      conv3x3_wino(th2, [(10, 4)],
                                   tp2, 0, twrU, tbr1, True, 0, vpool)
                      out_head(th2, 14 * 200, 0, s * 2800)

                  # 2-stage software pipeline: rpn/head lag the tr conv by
                  # two s-chunks so V-transforms hide under PE matmul work.
                  with tc.tile_pool(name="c2chunk", bufs=3) as c2pool:
                      pend = []
                      for s in range(4):
                          for ci in range(9):
                              r0 = ci * 2
                              tcc = c2pool.tile([128, 2, 2, 200], BF16,
                                                tag="c2")
                              nc.sync.dma_start(
                                  out=tcc[:],
                                  in_=c_d[0][:, :,
                                             14 * s + r0:14 * s + r0 + 2,
                                             :].rearrange(
                                      "k p r x -> p k r x"))
                              ps = pspool.tile([128, 2, 512], F32,
                                               tag="ps")
                              for mo in range(2):
                                  for kh in range(2):
                                      rhs = tcc[:, kh].rearrange(
                                          "p r x -> p (r x)")
                                      nc.tensor.matmul(
                                          ps[:, mo, :400], twl2[:, kh, mo],
                                          rhs,
                                          start=(kh == 0), stop=(kh == 1))
                                  ps4 = ps[:, mo, :400].rearrange(
                                      "p (r wh s) -> p r wh s",
                                      r=2, s=2)
                                  dest4 = tm2[:, mo, r0:r0 + 2, 1:201
                                              ].rearrange(
                                      "p r (wh s) -> p r wh s", s=2)
                                  srow = 1 + (14 * s + r0) // 2
                                  srcb = tm3[:, mo, srow, 1:101
                                             ].unsqueeze(1).unsqueeze(3)\
                                      .broadcast_to([128, 2, 100, 2])
                                  nc.vector.tensor_tensor(
                                      dest4, ps4, srcb, ALU.add)
                          if len(pend) >= 2:
                              rpn_head2(*pend.pop(0))
                          tp2 = p2pool.tile([128, 2, 16, 202], BF16,
                                            tag="p2")
                          zero_cols(tp2, 16, [0, 201])
                          conv3x3_wino(tp2, [(0, 4), (4, 4)],
                                       tm2, 0, twt2, tbtr[:, 0], False, 0,
                                       vpool)
                          conv3x3_wino(tp2, [(8, 4), (12, 4)],
                                       tm2, 0, twt2, tbtr[:, 0], False, 0,
                                       vpool)
                          mask_rows(tp2, MOFF["p2"] + 14 * s, [0, 15], 1, 201)
                          pend.append((s, tp2))
                      for item in pend:
                          rpn_head2(*item)

    nc.compile()
    return nc


def _prep_inputs(c2, c3, c4, c5, lat_w, tr_w, rpn_w1, rpn_b1, rpn_w2,
                 rpn_b2, lat_b, tr_b):
    import ml_dtypes
    bf16 = ml_dtypes.bfloat16
    cs = [c2, c3, c4, c5]
    base = {}

    def conv_w(w):
        # [O=256, I=256, 3, 3] -> [ih, 128k, mo, dy, dx, 128m]
        return np.ascontiguousarray(
            w.reshape(2, 128, 2, 128, 3, 3).transpose(2, 3, 0, 4, 5, 1)
        ).astype(bf16)

    G = np.array([[1, 0, 0], [0.5, 0.5, 0.5], [0.5, -0.5, 0.5], [0, 0, 1]],
                 np.float32)

    def conv_wU(w):
        # Winograd F(2,3) width: U[o,i,dy,t] = sum_dx G[t,dx] w[o,i,dy,dx]
        U = np.einsum("oiyx,tx->oiyt", w.astype(np.float32), G)
        return np.ascontiguousarray(
            U.reshape(2, 128, 2, 128, 3, 4).transpose(2, 3, 0, 4, 5, 1)
        ).astype(bf16)

    for i in range(4):
        base[f"wlat{i}"] = np.ascontiguousarray(
            lat_w[i].reshape(2, 128, KH[i], 128).transpose(2, 3, 0, 1)
        ).astype(bf16)
    for i in range(3):
        base[f"wtrU{i}"] = conv_wU(tr_w[i])
    base["wtr3"] = conv_w(tr_w[3])
    base["wrpn1"] = conv_w(rpn_w1)
    base["wrpnU"] = conv_wU(rpn_w1)
    w2 = np.zeros((2, 128, 16), np.float32)
    w2[:, :, :15] = rpn_w2.reshape(15, 2, 128).transpose(1, 2, 0)
    base["wrpn2"] = w2.astype(bf16)
    base["btr"] = np.ascontiguousarray(
        np.stack([b.reshape(2, 128).T for b in tr_b]).transpose(0, 1, 2))
    base["brpn1"] = np.ascontiguousarray(rpn_b1.reshape(2, 128).T)
    b2 = np.zeros((16, 1), np.float32)
    b2[:15, 0] = rpn_b2
    base["brpn2"] = b2
    base["zeros"] = np.zeros((128, 512), bf16)
    for b in lat_b:
        if np.abs(b).max() != 0:
            raise NotImplementedError("nonzero lateral bias not supported")

    in_maps = []
    starts = [S2, S3, S4, S5]
    m_off = [2, 2, 2, 3]
    for b in range(2):
        for q in range(4):
            m = dict(base)
            for i in range(4):
                r0 = starts[i][q] - m_off[i]
                rows = M_ROWS[i]
                sl = np.zeros((CIN[i], rows, CW[i]), np.float32)
                lo, hi = max(0, r0), min(H[i], r0 + rows)
                if hi > lo:
                    sl[:, lo - r0:hi - r0, :W[i]] = cs[i][b, :, lo:hi, :]
                m[f"c{i}"] = np.ascontiguousarray(
                    sl.reshape(KH[i], 128, rows, CW[i])).astype(bf16)
            mk = np.zeros(244, np.float32)
            spans = [(S2[q] - 2, 200, 60), (S2[q] - 1, 200, 58),
                     (S3[q] - 2, 100, 32), (S3[q] - 1, 100, 30),
                     (S4[q] - 2, 50, 18), (S4[q] - 1, 50, 16),
                     (S5[q] - 3, 25, 13), (S5[q] - 2, 25, 11),
                     (S6[q] - 1, 13, 6)]
            off = 0
            for a0, hh, ln in spans:
                for j in range(ln):
                    mk[off + j] = 1.0 if 0 <= a0 + j < hh else 0.0
                off += ln
            m["masks"] = np.broadcast_to(mk, (128, 244)).astype(bf16)
            in_maps.append(m)
    return in_maps


def _assemble(results):
    out = np.zeros((2, 159882, 5), np.float32)
    lvl_off = [0, 120000, 150000, 157500, 159375]
    starts = [S2, S3, S4, S5, S6]
    for b in range(2):
        for q in range(4):
            r = results[b * 4 + q]
            for lv in range(5):
                v, w = VALID[lv][q], W[lv]
                # device layout is [ch=15, pix]; transpose to pixel-major
                o = np.ascontiguousarray(
                    r[f"o{lv}"].reshape(15, OWN[lv] * w).T
                ).reshape(OWN[lv] * w * 3, 5)
                a0 = lvl_off[lv] + starts[lv][q] * w * 3
                out[b, a0:a0 + v * w * 3] = o[:v * w * 3]
    return out


def kernel(**inputs):
    import os
    from concourse.bass_utils import run_bass_kernel_spmd
    if "nc" not in _CACHED:
        _CACHED["nc"] = _build(int(os.environ.get("K_REPS", "1")))
    nc = _CACHED["nc"]
    in_maps = _prep_inputs(
        np.asarray(inputs["c2"], np.float32),
        np.asarray(inputs["c3"], np.float32),
        np.asarray(inputs["c4"], np.float32),
        np.asarray(inputs["c5"], np.float32),
        [np.asarray(inputs[f"lat_w{i}"], np.float32) for i in range(4)],
        [np.asarray(inputs[f"tr_w{i}"], np.float32) for i in range(4)],
        np.asarray(inputs["rpn_w1"], np.float32),
        np.asarray(inputs["rpn_b1"], np.float32),
        np.asarray(inputs["rpn_w2"], np.float32),
        np.asarray(inputs["rpn_b2"], np.float32),
        [np.asarray(inputs[f"lat_b{i}"], np.float32) for i in range(4)],
        [np.asarray(inputs[f"tr_b{i}"], np.float32) for i in range(4)],
    )
    res = run_bass_kernel_spmd(nc, in_maps, list(range(N_CORES)))
    return _assemble(res.results)
